# revision 1
# baseline (speedup 1.0000x reference)
"""DeepseekV3 decoder layer (MLA attention + dense MLP) on 8 trn2 NeuronCores.

Strategy: tensor-parallel in transposed-activation space ("T-space").
Activations are stored [feature, token] so every GEMM uses a natural-layout
weight shard as the PE stationary operand and 512-token chunks as the moving
operand (fp32r / bf16 at 1 cycle/row).  All cross-core movement is AllGather
(never AllReduce): each block's final GEMM is column-sharded and the output
is assembled on host from per-core column slices.

Per-core shards (prepared on host in kernel()):
  q_a/kv_a column shards; q_b/kv_b per-head column shards with columns
  reordered (nope|pe resp. k|v) so device rows stay 128-aligned; o/gate/up/
  down column shards.  hidden^T is passed replicated; the core's 512
  residual rows as a separate slice.

Collectives: AG(raw lqT), AG(raw lkvT), AG(attnT bf16), AG(h2T), AG(mT bf16).

All GEMMs run in bf16 (weights host-cast, activations rounded on the
PSUM->SBUF copy); PSUM accumulation, residual adds, softmax exp and norm
statistics stay fp32.  The rope rotation matmul uses an exact +-1 matrix.
"""
import sys

sys.path.insert(0, '/opt/trn_rl_repo')

import numpy as np
import ml_dtypes

S, D, H, QLORA, KVLORA = 1024, 4096, 32, 1536, 512
DN, DR, DV, INTER = 128, 64, 128, 11008
EPS = 1e-6
SCALE = (DN + DR) ** -0.5
NC = 8
HPC = H // NC               # 4 heads per core
QAC = QLORA // NC           # 192 q_a cols per core
KVAC = (KVLORA + DR) // NC  # 72 kv_a cols per core
OC = D // NC                # 512 o_proj/down cols per core
IC = INTER // NC            # 1376 gate/up cols per core

P = 128
TCH = 512                   # moving-operand chunk
NCH = S // TCH              # 2 token chunks
NDT = D // P                # 32
NKVT = KVLORA // P          # 4
NQLT = QLORA // P           # 12
NTT = S // P                # 8
NIT = INTER // P            # 86
NQB = HPC * (DN + DR) // P  # 6 qT row chunks
NOB = OC // P               # 4
BF16 = ml_dtypes.bfloat16

_CACHE = {}


def _build():
    import concourse.bass as bass
    import concourse.tile as tile
    from concourse import bacc, mybir
    from contextlib import ExitStack

    dt = mybir.dt
    f32, f32r, bf16 = dt.float32, dt.float32r, dt.bfloat16
    AF = mybir.ActivationFunctionType
    ts, ds = bass.ts, bass.ds

    nc = bacc.Bacc('TRN2', target_bir_lowering=False, debug=False,
                   num_devices=NC)

    hT = nc.dram_tensor('hT', [D, S], bf16, kind='ExternalInput')
    h_ownD = nc.dram_tensor('h_ownD', [OC, S], f32, kind='ExternalInput')
    qa_own = nc.dram_tensor('qa_own', [D, QAC], bf16, kind='ExternalInput')
    kva_own = nc.dram_tensor('kva_own', [D, KVAC], bf16, kind='ExternalInput')
    qb_own = nc.dram_tensor('qb_own', [QLORA, HPC * (DN + DR)], bf16, kind='ExternalInput')
    kvb_own = nc.dram_tensor('kvb_own', [KVLORA, HPC * (DN + DV)], bf16, kind='ExternalInput')
    o_own = nc.dram_tensor('o_own', [D, OC], bf16, kind='ExternalInput')
    gate_own = nc.dram_tensor('gate_own', [D, IC], bf16, kind='ExternalInput')
    up_own = nc.dram_tensor('up_own', [D, IC], bf16, kind='ExternalInput')
    down_own = nc.dram_tensor('down_own', [INTER, OC], bf16, kind='ExternalInput')
    cosT_d = nc.dram_tensor('cosT2', [P, S], f32, kind='ExternalInput')
    sinT_d = nc.dram_tensor('sinT2', [P, S], f32, kind='ExternalInput')
    rot2_d = nc.dram_tensor('rot2T', [P, P], bf16, kind='ExternalInput')
    masks_d = nc.dram_tensor('masks', [4, P, TCH], bf16, kind='ExternalInput')
    out = nc.dram_tensor('out', [OC, S], f32, kind='ExternalOutput')

    RG = [list(range(NC))]

    def mm(psum, lhsT, rhs, start, stop):
        nc.tensor.matmul(psum, lhsT, rhs, start=start, stop=stop)

    def mmb(psum, lhsT, rhs, start, stop):
        nc.tensor.matmul(psum, lhsT, rhs, start=start, stop=stop)

    with tile.TileContext(nc) as tc, ExitStack() as st:
        const = st.enter_context(tc.tile_pool(name='const', bufs=1))
        vecs = st.enter_context(tc.tile_pool(name='vecs', bufs=1))
        dram = st.enter_context(tc.tile_pool(name='dram', bufs=1, space='DRAM'))

        ones = const.tile([P, 1], f32)
        nc.vector.memset(ones, 1.0)
        ones_bf = const.tile([P, 1], bf16)
        nc.vector.memset(ones_bf, 1.0)
        ones_row = const.tile([1, P], f32)
        nc.vector.memset(ones_row, 1.0)
        eps1 = const.tile([1, 1], f32)
        nc.vector.memset(eps1, EPS)

        def bcast_row(row_ap, name, pool, ps_pool, bufs=1):
            """[1,S] SBUF -> [P,S] SBUF via ones-matmul broadcast."""
            ps = ps_pool.tile([P, S], f32, tag='bc_ps', bufs=1, name=f'{name}_ps')
            for c in range(NCH):
                mm(ps[:, ts(c, TCH)], ones_row, row_ap[0:1, ts(c, TCH)],
                   True, True)
            sb = pool.tile([P, S], f32, tag=f'{name}_bc', bufs=bufs, name=f'{name}_bc')
            nc.vector.tensor_copy(sb, ps)
            return sb

        def finish_norm(ps_sum, scale_meanN, name, extra_sq=None):
            """[1,S] PSUM sumsq -> [1,S] SBUF rsqrt(mean+eps) (optionally *r1^2)."""
            sb = vecs.tile([1, S], f32, tag=f'{name}_v', name=f'{name}_v')
            if extra_sq is not None:
                nc.vector.tensor_mul(sb, ps_sum, extra_sq)
            else:
                nc.vector.tensor_copy(sb, ps_sum)
            nc.scalar.activation(sb, sb, AF.Sqrt, bias=eps1, scale=scale_meanN)
            nc.vector.reciprocal(sb, sb)
            return sb

        lq_dram = dram.tile([QAC, S], bf16)
        lkv_dram = dram.tile([KVAC, S], bf16)
        lq_ag = dram.tile([QLORA, S], bf16, addr_space='Shared')
        lkv_ag = dram.tile([KVLORA + DR, S], bf16, addr_space='Shared')
        attn_dram = dram.tile([HPC * DV, S], bf16)
        attnT_ag = dram.tile([H * DV, S], bf16, addr_space='Shared')
        h2_dram = dram.tile([OC, S], bf16)
        h2_ag = dram.tile([D, S], bf16, addr_space='Shared')
        MA = 768                 # first 6 chunks of m (AG overlaps gate/up tail)
        MB = IC - MA             # 608
        m_dramA = dram.tile([MA, S], bf16)
        m_dramB = dram.tile([MB, S], bf16)
        m_agA = dram.tile([NC * MA, S], bf16, addr_space='Shared')
        m_agB = dram.tile([NC * MB, S], bf16, addr_space='Shared')

        # ============ phase 1: a-projections + input-norm stats ============
        with tc.tile_pool(name='ph1', bufs=3) as ph1, \
             tc.tile_pool(name='ph1ps', bufs=1, space='PSUM') as ph1ps:
            ps_lq = ph1ps.tile([P, S], f32, name='ps_lq')
            ps_lq2 = ph1ps.tile([QAC - P, S], f32, name='ps_lq2')
            ps_lkv = ph1ps.tile([KVAC, S], f32, name='ps_lkv')
            ps_ss1 = ph1ps.tile([1, S], f32, name='ps_ss1')
            G1 = 4
            wkva = ph1.tile([P, NDT, KVAC], bf16, tag='wkva', bufs=1, name='wkva')
            nc.sync.dma_start(out=wkva, in_=kva_own.rearrange('(k p) n -> p k n', p=P))
            for g in range(NDT // G1):
                hk4 = ph1.tile([P, G1, S], bf16, tag='hk4', name='hk4')
                nc.sync.dma_start(
                    out=hk4, in_=hT[g * G1 * P:(g + 1) * G1 * P, :]
                    .rearrange('(k p) s -> p k s', p=P))
                wq4 = ph1.tile([P, G1, QAC], bf16, tag='wq4', name='wq4')
                nc.sync.dma_start(
                    out=wq4, in_=qa_own[g * G1 * P:(g + 1) * G1 * P, :]
                    .rearrange('(k p) n -> p k n', p=P))
                for kk in range(G1):
                    k = g * G1 + kk
                    hk = hk4[:, kk, :]
                    sq = ph1.tile([P, S], bf16, tag='sq', name='sq')
                    nc.vector.tensor_mul(sq, hk, hk)
                    stt, spp = (k == 0), (k == NDT - 1)
                    for c in range(NCH):
                        cs = ts(c, TCH)
                        mm(ps_lq[:, cs], wq4[:, kk, 0:P], hk[:, cs], stt, spp)
                        mm(ps_lq2[:, cs], wq4[:, kk, P:QAC], hk[:, cs], stt, spp)
                        mm(ps_lkv[:, cs], wkva[:, k, :], hk[:, cs], stt, spp)
                        mm(ps_ss1[:, cs], ones_bf[:, 0:1], sq[:, cs], stt, spp)
            r1 = finish_norm(ps_ss1, 1.0 / D, 'r1')
            r1sq = vecs.tile([1, S], f32, name='r1sq')
            nc.vector.tensor_mul(r1sq, r1, r1)
            lq_sb = ph1.tile([P, S], bf16, tag='lq_sb', name='lq_sb')
            nc.vector.tensor_copy(lq_sb, ps_lq)
            nc.sync.dma_start(out=lq_dram[0:P, :], in_=lq_sb)
            lq_sb2 = ph1.tile([QAC - P, S], bf16, tag='lq_sb2', name='lq_sb2')
            nc.vector.tensor_copy(lq_sb2, ps_lq2)
            nc.sync.dma_start(out=lq_dram[P:QAC, :], in_=lq_sb2)
            lkv_sb = ph1.tile([KVAC, S], bf16, tag='lkv_sb', name='lkv_sb')
            nc.vector.tensor_copy(lkv_sb, ps_lkv)
            nc.sync.dma_start(out=lkv_dram[:], in_=lkv_sb)
        nc.gpsimd.collective_compute('AllGather', mybir.AluOpType.bypass,
                                     replica_groups=RG, ins=[lq_dram[:]], outs=[lq_ag[:]])
        nc.gpsimd.collective_compute('AllGather', mybir.AluOpType.bypass,
                                     replica_groups=RG, ins=[lkv_dram[:]], outs=[lkv_ag[:]])

        # pools living through attention
        with ExitStack() as att_st:
            att = att_st.enter_context(tc.tile_pool(name='att', bufs=1))
            qT = att.tile([P, NQB, S], bf16, name='qT')
            kT = att.tile([P, HPC, S], bf16, name='kT')
            v_sb = att.tile([P, NTT, HPC * DV], bf16, name='v_sb')
            kpe = att.tile([P, S], bf16, name='kpe')   # roped k_pe, both halves
            cos_sb = att.tile([P, S], f32, name='cos_sb')
            nc.sync.dma_start(out=cos_sb, in_=cosT_d[:])
            sin_sb = att.tile([P, S], f32, name='sin_sb')
            nc.sync.dma_start(out=sin_sb, in_=sinT_d[:])
            rot2_sb = att.tile([P, P], bf16, name='rot2_sb')
            nc.sync.dma_start(out=rot2_sb, in_=rot2_d[:])
            masks_sb = att.tile([P, 4, TCH], bf16, name='masks_sb')
            nc.sync.dma_start(out=masks_sb, in_=masks_d.rearrange('m p c -> p m c'))

            pre_st = ExitStack()
            pre = pre_st.enter_context(tc.tile_pool(name='pre', bufs=1))
            lqn = pre.tile([P, NQLT, S], bf16, name='lqn')
            kvn = pre.tile([P, NKVT, S], bf16, name='kvn')

            # ============ phase 2: lq/lkv norms, rope k_pe ============
            with tc.tile_pool(name='ph2', bufs=3) as ph2, \
                 tc.tile_pool(name='ph2ps', bufs=1, space='PSUM') as ph2ps:
                ps_ssq = ph2ps.tile([1, S], f32, name='ps_ssq')
                ps_sskv = ph2ps.tile([1, S], f32, name='ps_sskv')
                nc.sync.dma_start(
                    out=lqn, in_=lq_ag.rearrange('(k p) s -> p k s', p=P))
                nc.sync.dma_start(
                    out=kvn, in_=lkv_ag[0:KVLORA, :].rearrange('(k p) s -> p k s', p=P))
                for k in range(NQLT):
                    sq = ph2.tile([P, S], bf16, tag='sq2', bufs=2, name='sq2')
                    nc.vector.tensor_mul(sq, lqn[:, k, :], lqn[:, k, :])
                    for c in range(NCH):
                        mm(ps_ssq[:, ts(c, TCH)], ones_bf[:, 0:1], sq[:, ts(c, TCH)],
                           k == 0, k == NQLT - 1)
                for k in range(NKVT):
                    sq = ph2.tile([P, S], bf16, tag='sq2', bufs=2, name='sq2')
                    nc.vector.tensor_mul(sq, kvn[:, k, :], kvn[:, k, :])
                    for c in range(NCH):
                        mm(ps_sskv[:, ts(c, TCH)], ones_bf[:, 0:1], sq[:, ts(c, TCH)],
                           k == 0, k == NKVT - 1)
                rq = finish_norm(ps_ssq, 1.0 / QLORA, 'rq', extra_sq=r1sq)
                rkv = finish_norm(ps_sskv, 1.0 / KVLORA, 'rkv', extra_sq=r1sq)
                fq = vecs.tile([1, S], f32, name='fq')
                nc.vector.tensor_mul(fq, rq, r1)
                fkv = vecs.tile([1, S], f32, name='fkv')
                nc.vector.tensor_mul(fkv, rkv, r1)
                fq_b = bcast_row(fq, 'fq', ph2, ph2ps)
                fkv_b = bcast_row(fkv, 'fkv', ph2, ph2ps)
                r1_b = bcast_row(r1, 'r1', ph2, ph2ps)
                for k in range(NQLT):
                    nc.vector.tensor_mul(lqn[:, k, :], lqn[:, k, :], fq_b)
                for k in range(NKVT):
                    nc.vector.tensor_mul(kvn[:, k, :], kvn[:, k, :], fkv_b)
                kpe_raw = ph2.tile([DR, S], bf16, tag='kpe_raw', bufs=1, name='kpe_raw')
                nc.sync.dma_start(out=kpe_raw, in_=lkv_ag[KVLORA:KVLORA + DR, :])
                nc.vector.tensor_mul(kpe_raw, kpe_raw, r1_b[0:DR, :])
                # rope: kpe = raw*cos + (R@raw)*sin  (R applied via matmul)
                ps_rot = ph2ps.tile([DR, S], f32, tag='rot_ps', name='rot_ps')
                for c in range(NCH):
                    cs = ts(c, TCH)
                    nc.tensor.matmul(ps_rot[:, cs], rot2_sb[0:DR, 0:DR],
                                     kpe_raw[:, cs], start=True, stop=True)
                rot_s = ph2.tile([DR, S], f32, tag='rot_s', bufs=1, name='rot_s')
                nc.vector.tensor_mul(rot_s, ps_rot, sin_sb[0:DR, :])
                nc.vector.tensor_mul(kpe[0:DR, :], kpe_raw, cos_sb[0:DR, :])
                nc.vector.tensor_add(kpe[0:DR, :], kpe[0:DR, :], rot_s)
                # duplicate into partitions 64:128 (DMA shifts partitions)
                nc.sync.dma_start(out=kpe[DR:P, :], in_=kpe[0:DR, :])

            # ============ phase 3: q_b -> qT ; kv_b -> kT, v ============
            with tc.tile_pool(name='ph3', bufs=3) as ph3, \
                 tc.tile_pool(name='ph3ps', bufs=2, space='PSUM') as ph3ps:
                for mc in range(NQB):
                    ps = ph3ps.tile([P, S], f32, tag='big_ps', name='qT_ps')
                    wq3 = ph3.tile([P, NQLT, P], bf16, tag='wq3', bufs=2, name='wq3')
                    nc.sync.dma_start(
                        out=wq3,
                        in_=qb_own[:, ts(mc, P)].rearrange('(k p) n -> p k n', p=P))
                    for k in range(NQLT):
                        for c in range(NCH):
                            mm(ps[:, ts(c, TCH)], wq3[:, k, :], lqn[:, k, ts(c, TCH)],
                               k == 0, k == NQLT - 1)
                    if mc < HPC * DN // P:
                        nc.vector.tensor_copy(qT[:, mc, :], ps)
                    else:
                        # pe chunk (2 heads x 64 rows): rope via rotation matmul
                        qraw = ph3.tile([P, S], bf16, tag='qraw', bufs=2, name='qraw')
                        nc.vector.tensor_copy(qraw, ps)
                        ps2 = ph3ps.tile([P, S], f32, tag='big_ps', name='rot_q_ps')
                        for c in range(NCH):
                            cs = ts(c, TCH)
                            nc.tensor.matmul(ps2[:, cs], rot2_sb, qraw[:, cs],
                                             start=True, stop=True)
                        rot_s = ph3.tile([P, S], f32, tag='rot_qs', bufs=2, name='rot_qs')
                        nc.vector.tensor_mul(rot_s, ps2, sin_sb)
                        nc.vector.tensor_mul(qT[:, mc, :], qraw, cos_sb)
                        nc.vector.tensor_add(qT[:, mc, :], qT[:, mc, :], rot_s)
                for j in range(HPC):
                    ps = ph3ps.tile([P, S], f32, tag='big_ps', name='kT_ps')
                    wk3 = ph3.tile([P, NKVT, P], bf16, tag='wk3', bufs=2, name='wk3')
                    nc.sync.dma_start(
                        out=wk3,
                        in_=kvb_own[:, ts(j, DN)].rearrange('(k p) n -> p k n', p=P))
                    for k in range(NKVT):
                        for c in range(NCH):
                            mm(ps[:, ts(c, TCH)], wk3[:, k, :], kvn[:, k, ts(c, TCH)],
                               k == 0, k == NKVT - 1)
                    nc.vector.tensor_copy(kT[:, j, :], ps)
                vw = ph3.tile([P, NKVT, HPC * DV], bf16, tag='vw', bufs=1, name='vw')
                nc.sync.dma_start(
                    out=vw, in_=kvb_own[:, HPC * DN:].rearrange('(k p) n -> p k n', p=P))
                for i in range(NTT):
                    ps = ph3ps.tile([P, HPC * DV], f32, tag='v_ps', name='v_ps')
                    for k in range(NKVT):
                        mm(ps, kvn[:, k, ts(i, P)], vw[:, k, :], k == 0, k == NKVT - 1)
                    nc.vector.tensor_copy(v_sb[:, i, :], ps)
            pre_st.close()   # free lqn/kvn before attention

        # ============ phase 4: attention per head ============
            with tc.tile_pool(name='ph4', bufs=2) as ph4, \
                 tc.tile_pool(name='ph4p', bufs=2) as ph4p, \
                 tc.tile_pool(name='ph4ps', bufs=2, space='PSUM') as ph4ps:
                for j in range(HPC):
                    pe_mc = HPC * DN // P + (j * DR) // P
                    pe_off = (j * DR) % P
                    probs = []
                    for i in range(NTT):
                        row = []
                        for jq in range(NCH):
                            if jq < i // 4:
                                row.append(None)
                                continue
                            cs = ts(jq, TCH)
                            ps = ph4ps.tile([P, TCH], f32, tag='sc_ps', bufs=2,
                                            name='sc_ps')
                            mm(ps, kT[:, j, ts(i, P)], qT[:, j, cs], True, False)
                            mm(ps, kpe[pe_off:pe_off + DR, ts(i, P)],
                               qT[pe_off:pe_off + DR, pe_mc, cs], False, True)
                            e = ph4p.tile([P, TCH], bf16, tag=f'probs{i}', bufs=2,
                                          name=f'probs{i}_{jq}')
                            nc.scalar.activation(e, ps, AF.Exp, scale=SCALE)
                            if jq == i // 4:
                                nc.vector.tensor_mul(e, e, masks_sb[:, i % 4, :])
                            row.append(e)
                        probs.append(row)
                    ps_se = ph4ps.tile([1, S], f32, tag='se_ps', bufs=1, name='se_ps')
                    for jq in range(NCH):
                        cs = ts(jq, TCH)
                        valid = [i for i in range(NTT) if jq >= i // 4]
                        for n, i in enumerate(valid):
                            mmb(ps_se[:, cs], ones_bf[:, 0:1], probs[i][jq],
                                n == 0, n == len(valid) - 1)
                    recip = vecs.tile([1, S], f32, tag='recip', name='recip')
                    nc.vector.reciprocal(recip, ps_se)
                    recip_b = bcast_row(recip, 'recip', ph4, ph4ps, bufs=2)
                    for jq in range(NCH):
                        cs = ts(jq, TCH)
                        ps = ph4ps.tile([P, TCH], f32, tag='at_ps', bufs=2, name='at_ps')
                        valid = [i for i in range(NTT) if jq >= i // 4]
                        for n, i in enumerate(valid):
                            mmb(ps, v_sb[:, i, ts(j, DV)], probs[i][jq],
                                n == 0, n == len(valid) - 1)
                        a = ph4.tile([P, TCH], bf16, tag='attn_o', name='attn_o')
                        nc.vector.tensor_mul(a, ps, recip_b[:, cs])
                        nc.sync.dma_start(out=attn_dram[ts(j, DV), cs], in_=a)
        nc.gpsimd.collective_compute('AllGather', mybir.AluOpType.bypass,
                                     replica_groups=RG, ins=[attn_dram[:]], outs=[attnT_ag[:]])

        # ============ phase 5: o_proj + residual ============
        h2own_pool = st.enter_context(tc.tile_pool(name='h2own', bufs=1))
        h2_own_sb = h2own_pool.tile([P, NOB, S], f32, name='h2_own_sb')
        with tc.tile_pool(name='ph5', bufs=3) as ph5, \
             tc.tile_pool(name='ph5r', bufs=1) as ph5r, \
             tc.tile_pool(name='ph5ps', bufs=1, space='PSUM') as ph5ps:
            att_rs = ph5r.tile([P, H * DV // P, S], bf16, name='att_rs')
            nc.sync.dma_start(out=att_rs,
                              in_=attnT_ag.rearrange('(k p) s -> p k s', p=P))
            ps_o = [ph5ps.tile([P, S], f32, tag=f'o_ps{m}', name=f'o_ps{m}')
                    for m in range(NOB)]
            G5 = 8
            for g in range(H * DV // P // G5):
                w8 = ph5.tile([P, G5, OC], bf16, tag='ow8', name='ow8')
                nc.sync.dma_start(
                    out=w8, in_=o_own[g * G5 * P:(g + 1) * G5 * P, :]
                    .rearrange('(k p) n -> p k n', p=P))
                for kk in range(G5):
                    k = g * G5 + kk
                    for mcc in range(NOB):
                        for c in range(NCH):
                            cs = ts(c, TCH)
                            mmb(ps_o[mcc][:, cs], w8[:, kk, ts(mcc, P)],
                                att_rs[:, k, cs], k == 0, k == H * DV // P - 1)
            for mcc in range(NOB):
                hres = ph5.tile([P, S], f32, tag='hres', name='hres')
                nc.sync.dma_start(out=hres, in_=h_ownD[ts(mcc, P), :])
                nc.vector.tensor_add(h2_own_sb[:, mcc, :], ps_o[mcc], hres)
                h2b = ph5.tile([P, S], bf16, tag='h2b', name='h2b')
                nc.vector.tensor_copy(h2b, h2_own_sb[:, mcc, :])
                nc.sync.dma_start(out=h2_dram[ts(mcc, P), :], in_=h2b)
        nc.gpsimd.collective_compute('AllGather', mybir.AluOpType.bypass,
                                     replica_groups=RG, ins=[h2_dram[:]], outs=[h2_ag[:]])

        # ============ phase 6: post-norm + gate/up -> m ============
        with ExitStack() as mlp_st:
            mlp_sb = mlp_st.enter_context(tc.tile_pool(name='mlp_sb', bufs=1))
            h2T = mlp_sb.tile([P, NDT, S], bf16, name='h2T')
            with tc.tile_pool(name='ph6a', bufs=2) as ph6a, \
                 tc.tile_pool(name='ph6aps', bufs=1, space='PSUM') as ph6aps:
                ps_ss2 = ph6aps.tile([1, S], f32, name='ps_ss2')
                nc.sync.dma_start(
                    out=h2T, in_=h2_ag.rearrange('(k p) s -> p k s', p=P))
                for k in range(NDT):
                    sq = ph6a.tile([P, S], bf16, tag='sq6', name='sq6')
                    nc.vector.tensor_mul(sq, h2T[:, k, :], h2T[:, k, :])
                    for c in range(NCH):
                        mm(ps_ss2[:, ts(c, TCH)], ones_bf[:, 0:1], sq[:, ts(c, TCH)],
                           k == 0, k == NDT - 1)
                r2 = finish_norm(ps_ss2, 1.0 / D, 'r2')
                r2_b = bcast_row(r2, 'r2', mlp_sb, ph6aps)

            with tc.tile_pool(name='ph6', bufs=2) as ph6, \
                 tc.tile_pool(name='ph6w', bufs=4) as ph6w, \
                 tc.tile_pool(name='ph6ps', bufs=2, space='PSUM') as ph6ps:
                NMC = (IC + P - 1) // P
                for mcc in range(NMC):
                    rows = min(P, IC - mcc * P)
                    ps_g = ph6ps.tile([P, S], f32, tag='g_ps', name='g_ps')
                    ps_u = ph6ps.tile([P, S], f32, tag='u_ps', name='u_ps')
                    wg = ph6w.tile([P, NDT, rows], bf16, tag='wg', bufs=2, name='wg')
                    nc.sync.dma_start(
                        out=wg, in_=gate_own[:, ds(mcc * P, rows)]
                        .rearrange('(k p) n -> p k n', p=P))
                    wu = ph6w.tile([P, NDT, rows], bf16, tag='wu', bufs=2, name='wu')
                    nc.sync.dma_start(
                        out=wu, in_=up_own[:, ds(mcc * P, rows)]
                        .rearrange('(k p) n -> p k n', p=P))
                    for k in range(NDT):
                        for c in range(NCH):
                            cs = ts(c, TCH)
                            mm(ps_g[0:rows, cs], wg[:, k, :], h2T[:, k, cs],
                               k == 0, k == NDT - 1)
                            mm(ps_u[0:rows, cs], wu[:, k, :], h2T[:, k, cs],
                               k == 0, k == NDT - 1)
                    g = ph6.tile([P, S], f32, tag='g_sb', name='g_sb')
                    nc.vector.tensor_mul(g[0:rows], ps_g[0:rows], r2_b[0:rows])
                    nc.scalar.activation(g[0:rows], g[0:rows], AF.Silu)
                    u = ph6.tile([P, S], f32, tag='u_sb', name='u_sb')
                    nc.vector.tensor_mul(u[0:rows], ps_u[0:rows], r2_b[0:rows])
                    m = ph6.tile([P, S], bf16, tag='m_sb', name='m_sb')
                    nc.vector.tensor_mul(m[0:rows], g[0:rows], u[0:rows])
                    if mcc * P < MA:
                        nc.sync.dma_start(out=m_dramA[ds(mcc * P, rows), :],
                                          in_=m[0:rows])
                    else:
                        nc.sync.dma_start(out=m_dramB[ds(mcc * P - MA, rows), :],
                                          in_=m[0:rows])
        nc.gpsimd.collective_compute('AllGather', mybir.AluOpType.bypass,
                                     replica_groups=RG, ins=[m_dramA[:]], outs=[m_agA[:]])
        nc.gpsimd.collective_compute('AllGather', mybir.AluOpType.bypass,
                                     replica_groups=RG, ins=[m_dramB[:]], outs=[m_agB[:]])

        # ============ phase 7: down_proj + final residual ============
        with tc.tile_pool(name='ph7', bufs=4) as ph7, \
             tc.tile_pool(name='ph7ps', bufs=1, space='PSUM') as ph7ps:
            ps_d = [ph7ps.tile([P, S], f32, tag=f'd_ps{m}', name=f'd_ps{m}')
                    for m in range(NOB)]
            G7 = 2
            NTA = NC * MA // P       # 48 k-tiles in half A
            kglob = 0
            woff = 0
            for src_ag, ntiles in ((m_agA, NC * MA // P), (m_agB, NC * MB // P)):
                for g in range(ntiles // G7):
                    mk = ph7.tile([P, G7, S], bf16, tag='mk', name='mk')
                    nc.sync.dma_start(
                        out=mk, in_=src_ag[g * G7 * P:(g + 1) * G7 * P, :]
                        .rearrange('(k p) s -> p k s', p=P))
                    w = ph7.tile([P, G7, OC], bf16, tag='dw', name='dw')
                    nc.sync.dma_start(
                        out=w, in_=down_own[woff + g * G7 * P:woff + (g + 1) * G7 * P, :]
                        .rearrange('(k p) n -> p k n', p=P))
                    for kk in range(G7):
                        k = kglob + g * G7 + kk
                        for mcc in range(NOB):
                            for c in range(NCH):
                                cs = ts(c, TCH)
                                mmb(ps_d[mcc][:, cs], w[:, kk, ts(mcc, P)],
                                    mk[:, kk, cs], k == 0, k == NIT - 1)
                kglob += ntiles
                woff += ntiles * P
            for mcc in range(NOB):
                o = ph7.tile([P, S], f32, tag='o_out', name='o_out')
                nc.vector.tensor_add(o, ps_d[mcc], h2_own_sb[:, mcc, :])
                nc.sync.dma_start(out=out[ts(mcc, P), :], in_=o)

    nc.compile()
    return nc


def _prep_inputs(inputs):
    """Host-side sharding: returns list of 8 per-core input dicts."""
    h = np.ascontiguousarray(np.asarray(inputs['hidden_states'], np.float32))
    hT = np.ascontiguousarray(h.T)
    cosT = np.ascontiguousarray(np.asarray(inputs['cos'], np.float32).T)
    sinT = np.ascontiguousarray(np.asarray(inputs['sin'], np.float32).T)
    q_a_w = np.asarray(inputs['q_a_w'], np.float32)
    q_b_w = np.asarray(inputs['q_b_w'], np.float32)
    kv_a_w = np.asarray(inputs['kv_a_w'], np.float32)
    kv_b_w = np.asarray(inputs['kv_b_w'], np.float32)
    o_w = np.asarray(inputs['o_w'], np.float32)
    gate_w = np.asarray(inputs['gate_w'], np.float32)
    up_w = np.asarray(inputs['up_w'], np.float32)
    down_w = np.asarray(inputs['down_w'], np.float32)

    pidx = np.arange(P)[:, None]
    cidx = np.arange(TCH)[None, :]
    masks = np.stack([(cidx - pidx >= P * k) for k in range(4)]).astype(BF16)

    # cos/sin duplicated across both 64-partition halves
    cosT2 = np.ascontiguousarray(np.vstack([cosT, cosT]))
    sinT2 = np.ascontiguousarray(np.vstack([sinT, sinT]))
    # rotation matrix: rot(x) = R @ x with R[m, m+32] = -1, R[m+32, m] = +1
    # (per 64-row head block, two blocks stacked).  Passed as R2.T = lhsT.
    R = np.zeros((DR, DR), np.float32)
    R[np.arange(DR // 2), np.arange(DR // 2) + DR // 2] = -1.0
    R[np.arange(DR // 2) + DR // 2, np.arange(DR // 2)] = 1.0
    R2 = np.zeros((P, P), np.float32)
    R2[:DR, :DR] = R
    R2[DR:, DR:] = R
    rot2T = np.ascontiguousarray(R2.T)

    # down rows reordered to match the two-part m AllGather layout:
    # [rank-major rows 0:768 of each core's shard, then rows 768:1376]
    MA = 768
    m_row_order = np.concatenate(
        [np.arange(MA) + rr * IC for rr in range(NC)] +
        [np.arange(MA, IC) + rr * IC for rr in range(NC)])

    in_maps = []
    for r in range(NC):
        heads = range(r * HPC, (r + 1) * HPC)
        qb_cols = np.concatenate(
            [q_b_w[:, hh * (DN + DR):hh * (DN + DR) + DN] for hh in heads] +
            [q_b_w[:, hh * (DN + DR) + DN:(hh + 1) * (DN + DR)] for hh in heads],
            axis=1)
        kvb_cols = np.concatenate(
            [kv_b_w[:, hh * (DN + DV):hh * (DN + DV) + DN] for hh in heads] +
            [kv_b_w[:, hh * (DN + DV) + DN:(hh + 1) * (DN + DV)] for hh in heads],
            axis=1)
        in_maps.append({
            'hT': hT.astype(BF16),
            'h_ownD': np.ascontiguousarray(hT[r * OC:(r + 1) * OC]),
            'qa_own': np.ascontiguousarray(q_a_w[:, r * QAC:(r + 1) * QAC]).astype(BF16),
            'kva_own': np.ascontiguousarray(kv_a_w[:, r * KVAC:(r + 1) * KVAC]).astype(BF16),
            'qb_own': np.ascontiguousarray(qb_cols).astype(BF16),
            'kvb_own': np.ascontiguousarray(kvb_cols).astype(BF16),
            'o_own': np.ascontiguousarray(o_w[:, r * OC:(r + 1) * OC]).astype(BF16),
            'gate_own': np.ascontiguousarray(gate_w[:, r * IC:(r + 1) * IC]).astype(BF16),
            'up_own': np.ascontiguousarray(up_w[:, r * IC:(r + 1) * IC]).astype(BF16),
            'down_own': np.ascontiguousarray(
                down_w[m_row_order, r * OC:(r + 1) * OC]).astype(BF16),
            'cosT2': cosT2,
            'sinT2': sinT2,
            'rot2T': rot2T.astype(BF16),
            'masks': masks,
        })
    return in_maps


def kernel(**inputs) -> np.ndarray:
    if 'nc' not in _CACHE:
        _CACHE['nc'] = _build()
    nc = _CACHE['nc']
    from concourse.bass_utils import run_bass_kernel_spmd
    in_maps = _prep_inputs(inputs)
    res = run_bass_kernel_spmd(nc, in_maps, core_ids=list(range(NC)))
    outT = np.concatenate([res.results[r]['out'] for r in range(NC)], axis=0)
    return np.ascontiguousarray(outT.T)



# revision 22
# speedup vs baseline: 1.1215x; 1.1215x over previous
"""DeepseekV3 decoder layer (MLA attention + dense MLP) on 8 trn2 NeuronCores.

v2: token-chunk (512) pipelined tensor-parallel kernel in transposed
activation space ("T-space", activations [feature, token]).

Key differences vs v1:
- All AllGathers are split per 512-token chunk and overlapped with compute
  (lq/lkv/attn/h2 chunked; m kept in A/B row halves).
- RMSNorm statistics run on Scalar (square) + Vector (accumulate) engines;
  the PE only does a few [1,512] accumulating reduces per norm (was 208 M=1
  matmuls costing ~78us of PE).
- Norm scaling applied at PSUM drain (rsqrt factors commute with GEMMs), so
  GEMMs never wait for statistics.
- Broadcast [1,S]->[P,S] matmuls in bf16 (fp32 K=1 matmuls were 933ns each).
- Weights pre-laid-out host-side as [P, ktile*cols] flat blocks so every
  weight DMA is contiguous multi-KB rows.
"""
import sys

sys.path.insert(0, '/opt/trn_rl_repo')

import numpy as np
import ml_dtypes

S, D, H, QLORA, KVLORA = 1024, 4096, 32, 1536, 512
DN, DR, DV, INTER = 128, 64, 128, 11008
EPS = 1e-6
SCALE = (DN + DR) ** -0.5
NC = 8
HPC = H // NC               # 4 heads per core
QAC = QLORA // NC           # 192 q_a cols per core
KVAC = (KVLORA + DR) // NC  # 72 kv_a cols per core
OC = D // NC                # 512 o_proj/down cols per core
IC = INTER // NC            # 1376 gate/up cols per core

P = 128
TCH = 512                   # token chunk
NCH = S // TCH              # 2 chunks
NDT = D // P                # 32
NKVT = KVLORA // P          # 4
NQLT = QLORA // P           # 12
NTT = S // P                # 8
NIT = INTER // P            # 86
NQB = HPC * (DN + DR) // P  # 6 qT row chunks (4 nope + 2 pe)
NOB = OC // P               # 4
NMC = (IC + P - 1) // P     # 11 gate/up row tiles (last is 96)
MA = 768                    # m rows in AG half A (per core)
MB = IC - MA                # 608
BF16 = ml_dtypes.bfloat16

_CACHE = {}


def _build():
    import concourse.bass as bass
    import concourse.tile as tile
    from concourse import bacc, mybir
    from contextlib import ExitStack

    dt = mybir.dt
    f32, bf16 = dt.float32, dt.bfloat16
    AF = mybir.ActivationFunctionType
    ts, ds = bass.ts, bass.ds

    nc = bacc.Bacc('TRN2', target_bir_lowering=False, debug=False,
                   num_devices=NC)

    hT = nc.dram_tensor('hT', [P, NDT, S], bf16, kind='ExternalInput')
    h_ownD = nc.dram_tensor('h_ownD', [OC, S], f32, kind='ExternalInput')
    qa_own = nc.dram_tensor('qa_own', [P, NDT, QAC], bf16, kind='ExternalInput')
    kva_own = nc.dram_tensor('kva_own', [P, NDT, KVAC], bf16, kind='ExternalInput')
    qb_own = nc.dram_tensor('qb_own', [NQB * P, NQLT * P], bf16, kind='ExternalInput')
    kvb_own = nc.dram_tensor('kvb_own', [P, NKVT, HPC * (DN + DV)], bf16, kind='ExternalInput')
    o_own = nc.dram_tensor('o_own', [P, NDT * OC], bf16, kind='ExternalInput')
    gate_own = nc.dram_tensor('gate_own', [P, NDT * IC], bf16, kind='ExternalInput')
    up_own = nc.dram_tensor('up_own', [P, NDT * IC], bf16, kind='ExternalInput')
    down_own = nc.dram_tensor('down_own', [P, NIT * OC], bf16, kind='ExternalInput')
    cosT_d = nc.dram_tensor('cosT2', [P, S], f32, kind='ExternalInput')
    sinT_d = nc.dram_tensor('sinT2', [P, S], f32, kind='ExternalInput')
    rot2_d = nc.dram_tensor('rot2T', [P, P], bf16, kind='ExternalInput')
    masks_d = nc.dram_tensor('masks', [4, P, TCH], bf16, kind='ExternalInput')
    out = nc.dram_tensor('out', [OC, S], f32, kind='ExternalOutput')

    RG = [list(range(NC))]

    def mm(psum, lhsT, rhs, start, stop):
        nc.tensor.matmul(psum, lhsT, rhs, start=start, stop=stop)

    def ag(in_t, out_t):
        nc.gpsimd.collective_compute(
            'AllGather', mybir.AluOpType.bypass, replica_groups=RG,
            ins=[in_t[:]], outs=[out_t[:]])

    with tile.TileContext(nc) as tc, ExitStack() as st:
        const = st.enter_context(tc.tile_pool(name='const', bufs=1))
        vecs = st.enter_context(tc.tile_pool(name='vecs', bufs=1))
        dram = st.enter_context(tc.tile_pool(name='dram', bufs=1, space='DRAM'))

        ones_bf = const.tile([P, 1], bf16)
        nc.vector.memset(ones_bf, 1.0)
        onesrow_bf = const.tile([1, P], bf16)
        nc.vector.memset(onesrow_bf, 1.0)
        eps1 = const.tile([1, 1], f32)
        nc.vector.memset(eps1, EPS)

        lq_dram = [dram.tile([QAC, TCH], bf16, name=f'lq_dram{c}')
                   for c in range(NCH)]
        lq_ag = [dram.tile([QLORA, TCH], bf16, addr_space='Shared',
                           name=f'lq_ag{c}') for c in range(NCH)]
        lkv_dram = [dram.tile([KVAC, TCH], bf16, name=f'lkv_dram{c}')
                    for c in range(NCH)]
        lkv_ag = [dram.tile([KVLORA + DR, TCH], bf16, addr_space='Shared',
                            name=f'lkv_ag{c}') for c in range(NCH)]
        attn_dram = [dram.tile([HPC * DV, TCH], bf16, name=f'attn_dram{c}')
                     for c in range(NCH)]
        attn_ag = [dram.tile([H * DV, TCH], bf16, addr_space='Shared',
                             name=f'attn_ag{c}') for c in range(NCH)]
        h2_dram = [dram.tile([OC, TCH], bf16, name=f'h2_dram{c}')
                   for c in range(NCH)]
        h2_ag = [dram.tile([D, TCH], bf16, addr_space='Shared',
                           name=f'h2_ag{c}') for c in range(NCH)]
        m_dramA = dram.tile([MA, S], bf16)
        m_dramB = dram.tile([MB, S], bf16)
        m_agA = dram.tile([NC * MA, S], bf16, addr_space='Shared')
        m_agB = dram.tile([NC * MB, S], bf16, addr_space='Shared')

        # ---- helpers ----------------------------------------------------
        def vrow(name):
            return vecs.tile([1, TCH], f32, tag='vrow', bufs=4, name=name)

        def bcast_row(row_fp32, name, pool, ps_pool, ps_bufs=1, bufs=1):
            """[1,TCH] fp32 -> [P,TCH] fp32 SBUF (bf16 precision) via matmul."""
            rb = pool.tile([1, TCH], bf16, tag='brow', bufs=3, name=f'{name}_r')
            nc.vector.tensor_copy(rb, row_fp32)
            ps = ps_pool.tile([P, TCH], f32, tag='bc_ps', bufs=ps_bufs,
                              name=f'{name}_ps')
            mm(ps, onesrow_bf, rb[0:1, :], True, True)
            sb = pool.tile([P, TCH], f32, tag=f'{name}_bc', bufs=bufs,
                           name=f'{name}_bc')
            nc.vector.tensor_copy(sb, ps)
            return sb

        def finish_norm(ps_sum, scale_meanN, name, extra_sq=None, tag='vrow'):
            sb = vecs.tile([1, TCH], f32, tag=tag, bufs=4, name=name)
            if extra_sq is not None:
                nc.vector.tensor_mul(sb, ps_sum, extra_sq)
            else:
                nc.vector.tensor_copy(sb, ps_sum)
            nc.scalar.activation(sb, sb, AF.Sqrt, bias=eps1, scale=scale_meanN)
            nc.vector.reciprocal(sb, sb)
            return sb

        def sq_chains(get_src, n, width, pool, tag, nacc, k_lo=0, k_hi=None,
                      accs=None):
            """acc[a] accumulates get_src(k)^2 (ACT square + DVE adds)."""
            if k_hi is None:
                k_hi = n
            if accs is None:
                accs = [pool.tile([P, width], f32, tag=f'{tag}a{a}', bufs=1,
                                  name=f'{tag}a{a}') for a in range(nacc)]
            for k in range(k_lo, k_hi):
                a = k % nacc
                if k < nacc:
                    nc.scalar.activation(accs[a], get_src(k), AF.Square)
                else:
                    sq = pool.tile([P, width], f32, tag=f'{tag}s', bufs=2,
                                   name=f'{tag}s')
                    nc.scalar.activation(sq, get_src(k), AF.Square)
                    nc.vector.tensor_add(accs[a], accs[a], sq)
            return accs

        def sq_reduce(accs, cs, pool, ps_pool, tag, ps_bufs=1):
            ps = ps_pool.tile([1, TCH], f32, tag=tag, bufs=ps_bufs, name=tag)
            for a, acc in enumerate(accs):
                ab = pool.tile([P, TCH], bf16, tag='accb', bufs=2, name='accb')
                nc.vector.tensor_copy(ab, acc[:, cs] if cs is not None else acc)
                mm(ps, ones_bf, ab, a == 0, a == len(accs) - 1)
            return ps

        # ---- persistent SBUF --------------------------------------------
        resid = st.enter_context(tc.tile_pool(name='resid', bufs=1))
        h2_own_sb = resid.tile([P, NOB, S], f32, name='h2_own_sb')

        r1_c = [None, None]
        cosr1_c, sinr1_c = [None, None], [None, None]
        r2_b_c = [None, None]

        with ExitStack() as att_scope:
            attp = att_scope.enter_context(tc.tile_pool(name='attp', bufs=1))
            qT = attp.tile([P, NQB, S], bf16, name='qT')
            kT = attp.tile([P, HPC, S], bf16, name='kT')
            v_sb = attp.tile([P, NTT, HPC * DV], bf16, name='v_sb')
            kpe = attp.tile([P, S], bf16, name='kpe')
            cos_sb = attp.tile([P, S], f32, name='cos_sb')
            nc.sync.dma_start(out=cos_sb, in_=cosT_d[:])
            sin_sb = attp.tile([P, S], f32, name='sin_sb')
            nc.sync.dma_start(out=sin_sb, in_=sinT_d[:])
            rot2_sb = attp.tile([P, P], bf16, name='rot2_sb')
            nc.sync.dma_start(out=rot2_sb, in_=rot2_d[:])
            masks_sb = attp.tile([P, 4, TCH], bf16, name='masks_sb')
            nc.sync.dma_start(out=masks_sb, in_=masks_d.rearrange('m p c -> p m c'))
            qa_sb = attp.tile([P, NDT, QAC], bf16, name='qa_sb')
            nc.sync.dma_start(out=qa_sb, in_=qa_own[:])
            kva_sb = attp.tile([P, NDT, KVAC], bf16, name='kva_sb')
            nc.sync.dma_start(out=kva_sb, in_=kva_own[:])
            kvb_sb = attp.tile([P, NKVT, HPC * (DN + DV)], bf16, name='kvb_sb')
            nc.sync.dma_start(out=kvb_sb, in_=kvb_own[:])

            # ============ phase 1: a-projections + input-norm stats =======
            with ExitStack() as ph1_scope:
                hkp = ph1_scope.enter_context(tc.tile_pool(name='hkp', bufs=1))
                hk = hkp.tile([P, NDT, S], bf16, name='hk')
                G1 = 4
                for g in range(NDT // G1):
                    nc.sync.dma_start(out=hk[:, g * G1:(g + 1) * G1, :],
                                      in_=hT[:, g * G1:(g + 1) * G1, :])
                ph1 = ph1_scope.enter_context(tc.tile_pool(name='ph1', bufs=1))
                ph1ps = ph1_scope.enter_context(
                    tc.tile_pool(name='ph1ps', bufs=1, space='PSUM'))
                # lq GEMMs per chunk + AG  (PSUM: lq1 2 + lq2 1 = 3 banks)
                for c in range(NCH):
                    cs = ts(c, TCH)
                    ps1 = ph1ps.tile([P, TCH], f32, tag='lq1', bufs=2, name='lq1')
                    ps2 = ph1ps.tile([QAC - P, TCH], f32, tag='lq2', bufs=1,
                                     name='lq2')
                    for k in range(NDT):
                        mm(ps1, qa_sb[:, k, 0:P], hk[:, k, cs], k == 0, k == NDT - 1)
                        mm(ps2, qa_sb[:, k, P:QAC], hk[:, k, cs], k == 0, k == NDT - 1)
                    lq1 = ph1.tile([P, TCH], bf16, tag='lq1s', bufs=2, name='lq1s')
                    nc.vector.tensor_copy(lq1, ps1)
                    nc.sync.dma_start(out=lq_dram[c][0:P, :], in_=lq1)
                    lq2 = ph1.tile([QAC - P, TCH], bf16, tag='lq2s', bufs=2,
                                   name='lq2s')
                    nc.vector.tensor_copy(lq2, ps2)
                    nc.sync.dma_start(out=lq_dram[c][P:QAC, :], in_=lq2)
                    ag(lq_dram[c], lq_ag[c])
                    # first half of the square-accumulate chains fills DVE
                    # idle time before the first drain
                    if c == 0:
                        ss_accs = sq_chains(lambda k: hk[:, k, :], NDT, S, ph1,
                                            'ss1', 2, 0, 16)
                sq_chains(lambda k: hk[:, k, :], NDT, S, ph1, 'ss1', 2, 16, NDT,
                          accs=ss_accs)
                # lkv GEMMs per chunk + AG  (PSUM: 1 bank)
                for c in range(NCH):
                    cs = ts(c, TCH)
                    pskv = ph1ps.tile([KVAC, TCH], f32, tag='lkv', bufs=1,
                                      name='lkv')
                    for k in range(NDT):
                        mm(pskv, kva_sb[:, k, :], hk[:, k, cs], k == 0, k == NDT - 1)
                    lkv1 = ph1.tile([KVAC, TCH], bf16, tag='lkvs', bufs=2,
                                    name='lkvs')
                    nc.vector.tensor_copy(lkv1, pskv)
                    nc.sync.dma_start(out=lkv_dram[c][:], in_=lkv1)
                    ag(lkv_dram[c], lkv_ag[c])
                # input-norm factors per chunk (PSUM: ss 1 + bc 1 = 2 banks)
                for c in range(NCH):
                    cs = ts(c, TCH)
                    ss = sq_reduce(ss_accs, cs, ph1, ph1ps, 'ss_ps', ps_bufs=2)
                    r1 = finish_norm(ss, 1.0 / D, f'r1_{c}', tag='r1')
                    r1sq = vecs.tile([1, TCH], f32, tag='r1sq', bufs=2,
                                     name='r1sq')
                    nc.vector.tensor_mul(r1sq, r1, r1)
                    r1_c[c] = (r1, r1sq)
                    r1b = bcast_row(r1, f'r1_{c}', ph1, ph1ps, ps_bufs=2, bufs=1)
                    cr = attp.tile([P, TCH], bf16, tag='cosr1', bufs=2,
                                   name='cosr1')
                    nc.vector.tensor_mul(cr, cos_sb[:, cs], r1b)
                    sr = attp.tile([P, TCH], bf16, tag='sinr1', bufs=2,
                                   name='sinr1')
                    nc.vector.tensor_mul(sr, sin_sb[:, cs], r1b)
                    cosr1_c[c], sinr1_c[c] = cr, sr

            # o_proj weights resident; loaded during phases 2-4
            o_pool = att_scope.enter_context(tc.tile_pool(name='o_pool', bufs=1))
            o_sb = o_pool.tile([P, NDT, OC], bf16, name='o_sb')
            for g in range(4):
                nc.sync.dma_start(
                    out=o_sb[:, g * 8:(g + 1) * 8, :],
                    in_=o_own[:, ds(g * 8 * OC, 8 * OC)]
                    .rearrange('p (k n) -> p k n', n=OC))

            # ============ phases 2-4 per chunk ============================
            pre = att_scope.enter_context(tc.tile_pool(name='pre', bufs=1))
            for c in range(NCH):
                cs = ts(c, TCH)
                r1, r1sq = r1_c[c]
                with tc.tile_pool(name='ph2', bufs=1) as ph2, \
                     tc.tile_pool(name='ph2w', bufs=3) as ph2w, \
                     tc.tile_pool(name='ph2ps', bufs=1, space='PSUM') as ph2ps:
                    lqn = pre.tile([P, NQLT, TCH], bf16, tag='lqn', bufs=1,
                                   name='lqn')
                    for g in range(2):
                        nc.sync.dma_start(
                            out=lqn[:, g * 6:(g + 1) * 6, :],
                            in_=lq_ag[c].rearrange('(k p) s -> p k s', p=P)
                            [:, g * 6:(g + 1) * 6, :])
                    kvn = pre.tile([P, NKVT, TCH], bf16, tag='kvn', bufs=1,
                                   name='kvn')
                    nc.sync.dma_start(
                        out=kvn, in_=lkv_ag[c][0:KVLORA, :]
                        .rearrange('(k p) s -> p k s', p=P))

                    # q_b GEMM mc 0-2 (PSUM qb_ps: 3 banks)
                    def qbw_tile(mc2):
                        w = ph2w.tile([P, NQLT, P], bf16, tag='qbw', bufs=3,
                                      name='qbw')
                        nc.sync.dma_start(
                            out=w, in_=qb_own[ds(mc2 * P, P), :]
                            .rearrange('p (k n) -> p k n', n=P))
                        return w

                    ps_q = []
                    for mc2 in range(3):
                        w = qbw_tile(mc2)
                        ps = ph2ps.tile([P, TCH], f32, tag='qb_ps', bufs=3,
                                        name='qb_ps')
                        for k in range(NQLT):
                            mm(ps, w[:, k, :], lqn[:, k, :],
                               k == 0, k == NQLT - 1)
                        ps_q.append(ps)
                    # rq stats + fq factor (PSUM st 1 + bc 1 = 2 banks)
                    acc_q = sq_chains(lambda k: lqn[:, k, :], NQLT, TCH, ph2,
                                      'st2', 2)
                    ssq = sq_reduce(acc_q, None, ph2, ph2ps, 'st_ps')
                    rq = finish_norm(ssq, 1.0 / QLORA, f'rq_{c}', extra_sq=r1sq)
                    fq = vrow(f'fq_{c}')
                    nc.vector.tensor_mul(fq, rq, r1)
                    fq_b = bcast_row(fq, f'fq_{c}', ph2, ph2ps)
                    cf = ph2.tile([P, TCH], bf16, tag='cosfq', bufs=1,
                                  name='cosfq')
                    nc.vector.tensor_mul(cf, cos_sb[:, cs], fq_b)
                    sf = ph2.tile([P, TCH], bf16, tag='sinfq', bufs=1,
                                  name='sinfq')
                    nc.vector.tensor_mul(sf, sin_sb[:, cs], fq_b)
                    for mc2 in range(3):
                        nc.vector.tensor_mul(qT[:, mc2, cs], ps_q[mc2], fq_b)
                    # remaining q_b tiles: mc 3 (nope) + 4,5 (pe with rope)
                    # (PSUM qrot: 1 bank)
                    for mc2 in range(3, NQB):
                        w = qbw_tile(mc2)
                        ps = ph2ps.tile([P, TCH], f32, tag='qb_ps', bufs=3,
                                        name='qb_ps')
                        for k in range(NQLT):
                            mm(ps, w[:, k, :], lqn[:, k, :],
                               k == 0, k == NQLT - 1)
                        if mc2 == 3:
                            nc.vector.tensor_mul(qT[:, mc2, cs], ps, fq_b)
                        else:
                            qraw = ph2.tile([P, TCH], bf16, tag='qraw', bufs=1,
                                            name='qraw')
                            nc.vector.tensor_copy(qraw, ps)
                            ps2 = ph2ps.tile([P, TCH], f32, tag='qrot', bufs=1,
                                             name='qrot')
                            nc.tensor.matmul(ps2, rot2_sb, qraw,
                                             start=True, stop=True)
                            rot_s = ph2.tile([P, TCH], f32, tag='rot_qs',
                                             bufs=1, name='rot_qs')
                            nc.vector.tensor_mul(rot_s, ps2, sf)
                            nc.vector.tensor_mul(qT[:, mc2, cs], qraw, cf)
                            nc.vector.tensor_add(qT[:, mc2, cs],
                                                 qT[:, mc2, cs], rot_s)

                    # kv stats + fkv
                    acc_kv = sq_chains(lambda k: kvn[:, k, :], NKVT, TCH, ph2,
                                       'st2', 2)
                    sskv = sq_reduce(acc_kv, None, ph2, ph2ps, 'st_ps')
                    rkv = finish_norm(sskv, 1.0 / KVLORA, f'rkv_{c}',
                                      extra_sq=r1sq)
                    fkv = vrow(f'fkv_{c}')
                    nc.vector.tensor_mul(fkv, rkv, r1)
                    fkv_b = bcast_row(fkv, f'fkv_{c}', ph2, ph2ps)
                    # kT on raw kvn, drain-scaled (PSUM kv_ps: 2 banks)
                    for j in range(HPC):
                        ps = ph2ps.tile([P, TCH], f32, tag='kv_ps', bufs=2,
                                        name='kv_ps')
                        for k in range(NKVT):
                            mm(ps, kvb_sb[:, k, ts(j, DN)], kvn[:, k, :],
                               k == 0, k == NKVT - 1)
                        nc.vector.tensor_mul(kT[:, j, cs], ps, fkv_b)
                    # scale kvn in place for the v GEMM
                    for k in range(NKVT):
                        nc.vector.tensor_mul(kvn[:, k, :], kvn[:, k, :], fkv_b)
                    for i in range(4 * c, 4 * c + 4):
                        il = i - 4 * c
                        ps = ph2ps.tile([P, HPC * DV], f32, tag='kv_ps', bufs=2,
                                        name='kv_ps')
                        for k in range(NKVT):
                            mm(ps, kvn[:, k, ts(il, P)],
                               kvb_sb[:, k, HPC * DN:], k == 0, k == NKVT - 1)
                        nc.vector.tensor_copy(v_sb[:, i, :], ps)
                    # k_pe rope: kpe = raw*(cos*r1) + rot(raw)*(sin*r1)
                    kpe_raw = ph2.tile([DR, TCH], bf16, tag='kpe_raw', bufs=1,
                                       name='kpe_raw')
                    nc.sync.dma_start(out=kpe_raw,
                                      in_=lkv_ag[c][KVLORA:KVLORA + DR, :])
                    ps_rot = ph2ps.tile([P, TCH], f32, tag='qrot', bufs=1,
                                        name='kperot')
                    nc.tensor.matmul(ps_rot[0:DR, :], rot2_sb[0:DR, 0:DR],
                                     kpe_raw, start=True, stop=True)
                    rot_s = ph2.tile([DR, TCH], f32, tag='kpe_rs', bufs=1,
                                     name='kpe_rs')
                    nc.vector.tensor_mul(rot_s, ps_rot[0:DR, :],
                                         sinr1_c[c][0:DR, :])
                    kpe_t = ph2.tile([DR, TCH], f32, tag='kpe_t', bufs=1,
                                     name='kpe_t')
                    nc.vector.tensor_mul(kpe_t, kpe_raw, cosr1_c[c][0:DR, :])
                    nc.vector.tensor_add(kpe[0:DR, cs], kpe_t, rot_s)
                    nc.sync.dma_start(out=kpe[DR:P, cs], in_=kpe[0:DR, cs])

                # ---- attention for this chunk ----------------------------
                # PSUM: sc 2 + se 2 + at 2 + bc 2 = 8 banks
                with tc.tile_pool(name='ph4', bufs=1) as ph4, \
                     tc.tile_pool(name='ph4p', bufs=1) as ph4p, \
                     tc.tile_pool(name='ph4ps', bufs=1, space='PSUM') as ph4ps:
                    ilist = list(range(4 * c + 4))
                    for j in range(HPC):
                        pe_mc = HPC * DN // P + (j * DR) // P
                        pe_off = (j * DR) % P
                        probs = []
                        for i in ilist:
                            ps = ph4ps.tile([P, TCH], f32, tag='sc_ps', bufs=2,
                                            name='sc_ps')
                            mm(ps, kT[:, j, ts(i, P)], qT[:, j, cs], True, False)
                            mm(ps, kpe[pe_off:pe_off + DR, ts(i, P)],
                               qT[pe_off:pe_off + DR, pe_mc, cs], False, True)
                            e = ph4p.tile([P, TCH], bf16, tag=f'probs{i}',
                                          bufs=1, name=f'probs{i}')
                            nc.scalar.activation(e, ps, AF.Exp, scale=SCALE)
                            if i // 4 == c:
                                nc.vector.tensor_mul(e, e, masks_sb[:, i % 4, :])
                            probs.append(e)
                        ps_se = ph4ps.tile([1, TCH], f32, tag='se_ps', bufs=2,
                                           name='se_ps')
                        for n, i in enumerate(ilist):
                            mm(ps_se, ones_bf, probs[n], n == 0,
                               n == len(ilist) - 1)
                        ps_at = ph4ps.tile([P, TCH], f32, tag='at_ps', bufs=2,
                                           name='at_ps')
                        for n, i in enumerate(ilist):
                            mm(ps_at, v_sb[:, i, ts(j, DV)], probs[n],
                               n == 0, n == len(ilist) - 1)
                        recip = vrow(f'recip_{c}_{j}')
                        nc.vector.reciprocal(recip, ps_se)
                        recip_b = bcast_row(recip, 'recip', ph4, ph4ps,
                                            ps_bufs=2, bufs=1)
                        a = ph4.tile([P, TCH], bf16, tag='attn_o', bufs=2,
                                     name='attn_o')
                        nc.vector.tensor_mul(a, ps_at, recip_b)
                        nc.sync.dma_start(out=attn_dram[c][ts(j, DV), :], in_=a)
                ag(attn_dram[c], attn_ag[c])

            # ============ phase 5: o_proj + residual, per chunk ============
            with tc.tile_pool(name='ph5', bufs=1) as ph5, \
                 tc.tile_pool(name='ph5r', bufs=1) as ph5r, \
                 tc.tile_pool(name='ph5ps', bufs=1, space='PSUM') as ph5ps:
                hres_sb = ph5r.tile([P, NOB, S], f32, name='hres_sb')
                nc.sync.dma_start(out=hres_sb,
                                  in_=h_ownD.rearrange('(m p) s -> p m s', p=P))
                for c in range(NCH):
                    cs = ts(c, TCH)
                    att_rs = ph5r.tile([P, NDT, TCH], bf16, tag='att_rs', bufs=1,
                                       name='att_rs')
                    for g in range(4):
                        nc.sync.dma_start(
                            out=att_rs[:, g * 8:(g + 1) * 8, :],
                            in_=attn_ag[c].rearrange('(k p) s -> p k s', p=P)
                            [:, g * 8:(g + 1) * 8, :])
                    ps_o = [ph5ps.tile([P, TCH], f32, tag=f'o_ps{m2}', bufs=1,
                                       name=f'o_ps{m2}') for m2 in range(NOB)]
                    for k in range(NDT):
                        for mcc in range(NOB):
                            mm(ps_o[mcc], o_sb[:, k, ts(mcc, P)], att_rs[:, k, :],
                               k == 0, k == NDT - 1)
                    for mcc in range(NOB):
                        nc.vector.tensor_add(h2_own_sb[:, mcc, cs], ps_o[mcc],
                                             hres_sb[:, mcc, cs])
                        h2b = ph5.tile([P, TCH], bf16, tag='h2b', bufs=2,
                                       name='h2b')
                        nc.vector.tensor_copy(h2b, h2_own_sb[:, mcc, cs])
                        nc.sync.dma_start(out=h2_dram[c][ts(mcc, P), :], in_=h2b)
                    ag(h2_dram[c], h2_ag[c])

        # ============ phase 6: post-norm stats + gate/up (lag pipeline) ====
        with ExitStack() as mlp_scope:
            mlp_sb = mlp_scope.enter_context(tc.tile_pool(name='mlp_sb', bufs=1))
            h2T = mlp_sb.tile([P, NDT, S], bf16, name='h2T')
            ph6 = mlp_scope.enter_context(tc.tile_pool(name='ph6', bufs=1))
            ph6w = mlp_scope.enter_context(tc.tile_pool(name='ph6w', bufs=1))
            ph6ps = mlp_scope.enter_context(
                tc.tile_pool(name='ph6ps', bufs=1, space='PSUM'))
            for c in range(NCH):
                cs = ts(c, TCH)
                for g in range(4):
                    nc.sync.dma_start(
                        out=h2T[:, g * 8:(g + 1) * 8, cs],
                        in_=h2_ag[c].rearrange('(k p) s -> p k s', p=P)
                        [:, g * 8:(g + 1) * 8, :])

            def stats6(c):
                cs = ts(c, TCH)
                acc2 = sq_chains(lambda k: h2T[:, k, cs], NDT, TCH, ph6,
                                 'ss2', 2)
                ss2 = sq_reduce(acc2, None, ph6, ph6ps, 'st_ps')
                r2 = finish_norm(ss2, 1.0 / D, f'r2_{c}')
                r2_b_c[c] = bcast_row(r2, f'r2_{c}', mlp_sb, ph6ps)

            stats6(0)
            # job order: 3-tile c0 prologue, then interleave c1 at lag 2
            LAG = 2
            jobs = []
            for mcc in range(NMC):
                jobs.append((mcc, 0))
                if mcc >= LAG:
                    jobs.append((mcc - LAG, 1))
            for mcc in range(NMC - LAG, NMC):
                jobs.append((mcc, 1))
            woff = [mcc * NDT * P for mcc in range(NMC)]  # col offsets (els)
            wtiles = {}
            for mcc, c in jobs:
                if c == 1 and r2_b_c[1] is None:
                    stats6(1)
                cs = ts(c, TCH)
                rows = min(P, IC - mcc * P)
                if mcc not in wtiles:
                    wg = ph6w.tile([P, NDT, P], bf16, tag='wg', bufs=3,
                                   name='wg')
                    nc.sync.dma_start(
                        out=wg[:, :, 0:rows],
                        in_=gate_own[:, ds(woff[mcc], NDT * rows)]
                        .rearrange('p (k n) -> p k n', n=rows))
                    wu = ph6w.tile([P, NDT, P], bf16, tag='wu', bufs=3,
                                   name='wu')
                    nc.sync.dma_start(
                        out=wu[:, :, 0:rows],
                        in_=up_own[:, ds(woff[mcc], NDT * rows)]
                        .rearrange('p (k n) -> p k n', n=rows))
                    wtiles[mcc] = (wg, wu)
                wg, wu = wtiles[mcc]
                ps_g = ph6ps.tile([P, TCH], f32, tag='g_ps', bufs=3, name='g_ps')
                ps_u = ph6ps.tile([P, TCH], f32, tag='u_ps', bufs=3, name='u_ps')
                for k in range(NDT):
                    mm(ps_g[0:rows], wg[:, k, 0:rows], h2T[:, k, cs],
                       k == 0, k == NDT - 1)
                    mm(ps_u[0:rows], wu[:, k, 0:rows], h2T[:, k, cs],
                       k == 0, k == NDT - 1)
                g = ph6.tile([P, TCH], f32, tag='g_sb', bufs=2, name='g_sb')
                nc.vector.tensor_mul(g[0:rows], ps_g[0:rows],
                                     r2_b_c[c][0:rows])
                nc.scalar.activation(g[0:rows], g[0:rows], AF.Silu)
                u = ph6.tile([P, TCH], f32, tag='u_sb', bufs=2, name='u_sb')
                nc.vector.tensor_mul(u[0:rows], ps_u[0:rows],
                                     r2_b_c[c][0:rows])
                m = ph6.tile([P, TCH], bf16, tag='m_sb', bufs=2, name='m_sb')
                nc.vector.tensor_mul(m[0:rows], g[0:rows], u[0:rows])
                if mcc * P < MA:
                    nc.sync.dma_start(out=m_dramA[ds(mcc * P, rows), cs],
                                      in_=m[0:rows])
                else:
                    nc.sync.dma_start(out=m_dramB[ds(mcc * P - MA, rows), cs],
                                      in_=m[0:rows])
                if (mcc, c) == (MA // P - 1, 1):
                    ag(m_dramA, m_agA)
            ag(m_dramB, m_agB)

        # ============ phase 7: down_proj + final residual ============
        with tc.tile_pool(name='ph7', bufs=1) as ph7, \
             tc.tile_pool(name='ph7ps', bufs=1, space='PSUM') as ph7ps:
            ps_d = [ph7ps.tile([P, S], f32, tag=f'd_ps{m2}', bufs=1,
                               name=f'd_ps{m2}') for m2 in range(NOB)]
            G7 = 2
            kglob = 0
            for half, (src_ag, ntiles) in enumerate(
                    ((m_agA, NC * MA // P), (m_agB, NC * MB // P))):
                for g in range(ntiles // G7):
                    mk = ph7.tile([P, G7, S], bf16, tag='mk', bufs=4, name='mk')
                    nc.sync.dma_start(
                        out=mk, in_=src_ag[g * G7 * P:(g + 1) * G7 * P, :]
                        .rearrange('(k p) s -> p k s', p=P))
                    w = ph7.tile([P, G7, OC], bf16, tag='dw', bufs=4, name='dw')
                    nc.sync.dma_start(
                        out=w, in_=down_own[:, ds((kglob + g * G7) * OC, G7 * OC)]
                        .rearrange('p (k n) -> p k n', n=OC))
                    for kk in range(G7):
                        k = kglob + g * G7 + kk
                        for mcc in range(NOB):
                            for cc in range(NCH):
                                cc_s = ts(cc, TCH)
                                mm(ps_d[mcc][:, cc_s], w[:, kk, ts(mcc, P)],
                                   mk[:, kk, cc_s], k == 0, k == NIT - 1)
                kglob += ntiles
            for mcc in range(NOB):
                o = ph7.tile([P, S], f32, tag='o_out', bufs=2, name='o_out')
                nc.vector.tensor_add(o, ps_d[mcc], h2_own_sb[:, mcc, :])
                nc.sync.dma_start(out=out[ts(mcc, P), :], in_=o)

    nc.compile()
    return nc


def _tileize(w, cols_slice=None):
    """[D_in, n] -> [P, D_in//P, n] contiguous bf16."""
    if cols_slice is not None:
        w = w[:, cols_slice]
    kin = w.shape[0] // P
    return np.ascontiguousarray(
        w.reshape(kin, P, w.shape[1]).transpose(1, 0, 2)).astype(BF16)


def _flat(w, cols_slice=None):
    """[D_in, n] -> [P, (D_in//P)*n] flat k-major blocks."""
    t = _tileize(w, cols_slice)
    return np.ascontiguousarray(t.reshape(P, -1))


def _prep_inputs(inputs):
    h = np.ascontiguousarray(np.asarray(inputs['hidden_states'], np.float32))
    hT = np.ascontiguousarray(h.T)
    cosT = np.ascontiguousarray(np.asarray(inputs['cos'], np.float32).T)
    sinT = np.ascontiguousarray(np.asarray(inputs['sin'], np.float32).T)
    q_a_w = np.asarray(inputs['q_a_w'], np.float32)
    q_b_w = np.asarray(inputs['q_b_w'], np.float32)
    kv_a_w = np.asarray(inputs['kv_a_w'], np.float32)
    kv_b_w = np.asarray(inputs['kv_b_w'], np.float32)
    o_w = np.asarray(inputs['o_w'], np.float32)
    gate_w = np.asarray(inputs['gate_w'], np.float32)
    up_w = np.asarray(inputs['up_w'], np.float32)
    down_w = np.asarray(inputs['down_w'], np.float32)

    pidx = np.arange(P)[:, None]
    cidx = np.arange(TCH)[None, :]
    masks = np.stack([(cidx - pidx >= P * k) for k in range(4)]).astype(BF16)

    cosT2 = np.ascontiguousarray(np.vstack([cosT, cosT]))
    sinT2 = np.ascontiguousarray(np.vstack([sinT, sinT]))
    R = np.zeros((DR, DR), np.float32)
    R[np.arange(DR // 2), np.arange(DR // 2) + DR // 2] = -1.0
    R[np.arange(DR // 2) + DR // 2, np.arange(DR // 2)] = 1.0
    R2 = np.zeros((P, P), np.float32)
    R2[:DR, :DR] = R
    R2[DR:, DR:] = R
    rot2T = np.ascontiguousarray(R2.T)

    m_row_order = np.concatenate(
        [np.arange(MA) + rr * IC for rr in range(NC)] +
        [np.arange(MA, IC) + rr * IC for rr in range(NC)])

    hT_tiled = _tileize(hT)

    def gup_flat(w, r):
        """per-mc [P, NDT*rows] blocks concatenated along the free axis."""
        blocks = []
        for mcc in range(NMC):
            rows = min(P, IC - mcc * P)
            blocks.append(_flat(w, np.s_[r * IC + mcc * P:
                                         r * IC + mcc * P + rows]))
        return np.ascontiguousarray(np.concatenate(blocks, axis=1))

    in_maps = []
    for r in range(NC):
        heads = range(r * HPC, (r + 1) * HPC)
        qb_cols = np.concatenate(
            [q_b_w[:, hh * (DN + DR):hh * (DN + DR) + DN] for hh in heads] +
            [q_b_w[:, hh * (DN + DR) + DN:(hh + 1) * (DN + DR)] for hh in heads],
            axis=1)
        kvb_cols = np.concatenate(
            [kv_b_w[:, hh * (DN + DV):hh * (DN + DV) + DN] for hh in heads] +
            [kv_b_w[:, hh * (DN + DV) + DN:(hh + 1) * (DN + DV)] for hh in heads],
            axis=1)
        # q_b as [NQB*P, NQLT*P]: per-mc [P, NQLT*P] flat blocks stacked
        qb_blocks = np.stack([_flat(qb_cols, np.s_[mc * P:(mc + 1) * P])
                              for mc in range(NQB)])
        in_maps.append({
            'hT': hT_tiled,
            'h_ownD': np.ascontiguousarray(hT[r * OC:(r + 1) * OC]),
            'qa_own': _tileize(q_a_w, np.s_[r * QAC:(r + 1) * QAC]),
            'kva_own': _tileize(kv_a_w, np.s_[r * KVAC:(r + 1) * KVAC]),
            'qb_own': np.ascontiguousarray(
                qb_blocks.reshape(NQB * P, NQLT * P)),
            'kvb_own': _tileize(kvb_cols),
            'o_own': _flat(o_w, np.s_[r * OC:(r + 1) * OC]),
            'gate_own': gup_flat(gate_w, r),
            'up_own': gup_flat(up_w, r),
            'down_own': _flat(
                np.ascontiguousarray(down_w[m_row_order,
                                            r * OC:(r + 1) * OC])),
            'cosT2': cosT2,
            'sinT2': sinT2,
            'rot2T': rot2T.astype(BF16),
            'masks': masks,
        })
    return in_maps


def kernel(**inputs) -> np.ndarray:
    if 'nc' not in _CACHE:
        _CACHE['nc'] = _build()
    nc = _CACHE['nc']
    from concourse.bass_utils import run_bass_kernel_spmd
    in_maps = _prep_inputs(inputs)
    res = run_bass_kernel_spmd(nc, in_maps, core_ids=list(range(NC)))
    outT = np.concatenate([res.results[r]['out'] for r in range(NC)], axis=0)
    return np.ascontiguousarray(outT.T)


# revision 23
# speedup vs baseline: 1.2174x; 1.0856x over previous
"""DeepseekV3 decoder layer (MLA attention + dense MLP) on 8 trn2 NeuronCores.

v2: token-chunk (512) pipelined tensor-parallel kernel in transposed
activation space ("T-space", activations [feature, token]).

Key differences vs v1:
- All AllGathers are split per 512-token chunk and overlapped with compute
  (lq/lkv/attn/h2 chunked; m kept in A/B row halves).
- RMSNorm statistics run on Scalar (square) + Vector (accumulate) engines;
  the PE only does a few [1,512] accumulating reduces per norm (was 208 M=1
  matmuls costing ~78us of PE).
- Norm scaling applied at PSUM drain (rsqrt factors commute with GEMMs), so
  GEMMs never wait for statistics.
- Broadcast [1,S]->[P,S] matmuls in bf16 (fp32 K=1 matmuls were 933ns each).
- Weights pre-laid-out host-side as [P, ktile*cols] flat blocks so every
  weight DMA is contiguous multi-KB rows.
"""
import sys

sys.path.insert(0, '/opt/trn_rl_repo')

import numpy as np
import ml_dtypes

S, D, H, QLORA, KVLORA = 1024, 4096, 32, 1536, 512
DN, DR, DV, INTER = 128, 64, 128, 11008
EPS = 1e-6
SCALE = (DN + DR) ** -0.5
NC = 8
HPC = H // NC               # 4 heads per core
QAC = QLORA // NC           # 192 q_a cols per core
KVAC = (KVLORA + DR) // NC  # 72 kv_a cols per core
OC = D // NC                # 512 o_proj/down cols per core
IC = INTER // NC            # 1376 gate/up cols per core

P = 128
TCH = 512                   # token chunk
NCH = S // TCH              # 2 chunks
NDT = D // P                # 32
NKVT = KVLORA // P          # 4
NQLT = QLORA // P           # 12
NTT = S // P                # 8
NIT = INTER // P            # 86
NQB = HPC * (DN + DR) // P  # 6 qT row chunks (4 nope + 2 pe)
NOB = OC // P               # 4
NMC = (IC + P - 1) // P     # 11 gate/up row tiles (last is 96)
MA = 768                    # m rows in AG half A (per core)
MB = IC - MA                # 608
BF16 = ml_dtypes.bfloat16

_CACHE = {}


def _build():
    import concourse.bass as bass
    import concourse.tile as tile
    from concourse import bacc, mybir
    from contextlib import ExitStack

    dt = mybir.dt
    f32, bf16 = dt.float32, dt.bfloat16
    AF = mybir.ActivationFunctionType
    ts, ds = bass.ts, bass.ds

    nc = bacc.Bacc('TRN2', target_bir_lowering=False, debug=False,
                   num_devices=NC)

    hT = nc.dram_tensor('hT', [P, NDT, S], bf16, kind='ExternalInput')
    h_ownD = nc.dram_tensor('h_ownD', [OC, S], f32, kind='ExternalInput')
    qa_own = nc.dram_tensor('qa_own', [P, NDT, QAC], bf16, kind='ExternalInput')
    kva_own = nc.dram_tensor('kva_own', [P, NDT, KVAC], bf16, kind='ExternalInput')
    qb_own = nc.dram_tensor('qb_own', [NQB * P, NQLT * P], bf16, kind='ExternalInput')
    kvb_own = nc.dram_tensor('kvb_own', [P, NKVT, HPC * (DN + DV)], bf16, kind='ExternalInput')
    o_own = nc.dram_tensor('o_own', [P, NDT * OC], bf16, kind='ExternalInput')
    gate_own = nc.dram_tensor('gate_own', [P, NDT * IC], bf16, kind='ExternalInput')
    up_own = nc.dram_tensor('up_own', [P, NDT * IC], bf16, kind='ExternalInput')
    down_own = nc.dram_tensor('down_own', [P, NIT * OC], bf16, kind='ExternalInput')
    cosT_d = nc.dram_tensor('cosT2', [P, S], f32, kind='ExternalInput')
    sinT_d = nc.dram_tensor('sinT2', [P, S], f32, kind='ExternalInput')
    rot2_d = nc.dram_tensor('rot2T', [P, P], bf16, kind='ExternalInput')
    masks_d = nc.dram_tensor('masks', [4, P, TCH], bf16, kind='ExternalInput')
    out = nc.dram_tensor('out', [OC, S], f32, kind='ExternalOutput')

    RG = [list(range(NC))]

    def mm(psum, lhsT, rhs, start, stop):
        nc.tensor.matmul(psum, lhsT, rhs, start=start, stop=stop)

    def ag(in_t, out_t):
        nc.gpsimd.collective_compute(
            'AllGather', mybir.AluOpType.bypass, replica_groups=RG,
            ins=[in_t[:]], outs=[out_t[:]])

    with tile.TileContext(nc) as tc, ExitStack() as st:
        const = st.enter_context(tc.tile_pool(name='const', bufs=1))
        vecs = st.enter_context(tc.tile_pool(name='vecs', bufs=1))
        dram = st.enter_context(tc.tile_pool(name='dram', bufs=1, space='DRAM'))

        ones_bf = const.tile([P, 1], bf16)
        nc.vector.memset(ones_bf, 1.0)
        onesrow_bf = const.tile([1, P], bf16)
        nc.vector.memset(onesrow_bf, 1.0)
        eps1 = const.tile([1, 1], f32)
        nc.vector.memset(eps1, EPS)

        lq_dram = [dram.tile([QAC, TCH], bf16, name=f'lq_dram{c}')
                   for c in range(NCH)]
        lq_ag = [dram.tile([QLORA, TCH], bf16, addr_space='Shared',
                           name=f'lq_ag{c}') for c in range(NCH)]
        lkv_dram = [dram.tile([KVAC, TCH], bf16, name=f'lkv_dram{c}')
                    for c in range(NCH)]
        lkv_ag = [dram.tile([KVLORA + DR, TCH], bf16, addr_space='Shared',
                            name=f'lkv_ag{c}') for c in range(NCH)]
        attn_dram = [dram.tile([HPC * DV, TCH], bf16, name=f'attn_dram{c}')
                     for c in range(NCH)]
        attn_ag = [dram.tile([H * DV, TCH], bf16, addr_space='Shared',
                             name=f'attn_ag{c}') for c in range(NCH)]
        h2_dram = [dram.tile([OC, TCH], bf16, name=f'h2_dram{c}')
                   for c in range(NCH)]
        h2_ag = [dram.tile([D, TCH], bf16, addr_space='Shared',
                           name=f'h2_ag{c}') for c in range(NCH)]
        m_dramA = dram.tile([MA, S], bf16)
        m_dramB = dram.tile([MB, S], bf16)
        m_agA = dram.tile([NC * MA, S], bf16, addr_space='Shared')
        m_agB = dram.tile([NC * MB, S], bf16, addr_space='Shared')

        # ---- helpers ----------------------------------------------------
        def vrow(name):
            return vecs.tile([1, TCH], f32, tag='vrow', bufs=4, name=name)

        def bcast_row(row_fp32, name, pool, ps_pool, ps_bufs=1, bufs=1):
            """[1,TCH] fp32 -> [P,TCH] fp32 SBUF (bf16 precision) via matmul."""
            rb = pool.tile([1, TCH], bf16, tag='brow', bufs=3, name=f'{name}_r')
            nc.vector.tensor_copy(rb, row_fp32)
            ps = ps_pool.tile([P, TCH], f32, tag='bc_ps', bufs=ps_bufs,
                              name=f'{name}_ps')
            mm(ps, onesrow_bf, rb[0:1, :], True, True)
            sb = pool.tile([P, TCH], f32, tag=f'{name}_bc', bufs=bufs,
                           name=f'{name}_bc')
            nc.vector.tensor_copy(sb, ps)
            return sb

        def finish_norm(ps_sum, scale_meanN, name, extra_sq=None, tag='vrow'):
            sb = vecs.tile([1, TCH], f32, tag=tag, bufs=4, name=name)
            if extra_sq is not None:
                nc.vector.tensor_mul(sb, ps_sum, extra_sq)
            else:
                nc.vector.tensor_copy(sb, ps_sum)
            nc.scalar.activation(sb, sb, AF.Sqrt, bias=eps1, scale=scale_meanN)
            nc.vector.reciprocal_approx_fast(out=sb, in_=sb)
            return sb

        def sq_chains(get_src, n, width, pool, tag, nacc, k_lo=0, k_hi=None,
                      accs=None):
            """acc[a] accumulates get_src(k)^2 (ACT square + DVE adds)."""
            if k_hi is None:
                k_hi = n
            if accs is None:
                accs = [pool.tile([P, width], f32, tag=f'{tag}a{a}', bufs=1,
                                  name=f'{tag}a{a}') for a in range(nacc)]
            for k in range(k_lo, k_hi):
                a = k % nacc
                if k < nacc:
                    nc.scalar.activation(accs[a], get_src(k), AF.Square)
                else:
                    sq = pool.tile([P, width], f32, tag=f'{tag}s', bufs=2,
                                   name=f'{tag}s')
                    nc.scalar.activation(sq, get_src(k), AF.Square)
                    nc.vector.tensor_add(accs[a], accs[a], sq)
            return accs

        def sq_reduce(accs, cs, pool, ps_pool, tag, ps_bufs=1):
            ps = ps_pool.tile([1, TCH], f32, tag=tag, bufs=ps_bufs, name=tag)
            for a, acc in enumerate(accs):
                ab = pool.tile([P, TCH], bf16, tag='accb', bufs=2, name='accb')
                nc.vector.tensor_copy(ab, acc[:, cs] if cs is not None else acc)
                mm(ps, ones_bf, ab, a == 0, a == len(accs) - 1)
            return ps

        # warmup collective: pays the first-AG barrier/ramp cost during the
        # DMA-bound kernel start instead of on the critical path
        warm_in = dram.tile([P, 16], bf16, name='warm_in')
        warm_out = dram.tile([NC * P, 16], bf16, addr_space='Shared',
                             name='warm_out')
        warm_sb = const.tile([P, 16], bf16)
        nc.vector.memset(warm_sb, 0.0)
        nc.sync.dma_start(out=warm_in[:], in_=warm_sb)
        ag(warm_in, warm_out)

        # ---- persistent SBUF --------------------------------------------
        resid = st.enter_context(tc.tile_pool(name='resid', bufs=1))
        h2_own_sb = resid.tile([P, NOB, S], f32, name='h2_own_sb')

        r1_c = [None, None]
        cosr1_c, sinr1_c = [None, None], [None, None]
        r2_b_c = [None, None]

        with ExitStack() as att_scope:
            attp = att_scope.enter_context(tc.tile_pool(name='attp', bufs=1))
            qT = attp.tile([P, NQB, S], bf16, name='qT')
            kT = attp.tile([P, HPC, S], bf16, name='kT')
            v_sb = attp.tile([P, NTT, HPC * DV], bf16, name='v_sb')
            kpe = attp.tile([P, S], bf16, name='kpe')
            cos_sb = attp.tile([P, S], f32, name='cos_sb')
            nc.sync.dma_start(out=cos_sb, in_=cosT_d[:])
            sin_sb = attp.tile([P, S], f32, name='sin_sb')
            nc.sync.dma_start(out=sin_sb, in_=sinT_d[:])
            rot2_sb = attp.tile([P, P], bf16, name='rot2_sb')
            nc.sync.dma_start(out=rot2_sb, in_=rot2_d[:])
            masks_sb = attp.tile([P, 4, TCH], bf16, name='masks_sb')
            nc.sync.dma_start(out=masks_sb, in_=masks_d.rearrange('m p c -> p m c'))
            qa_sb = attp.tile([P, NDT, QAC], bf16, name='qa_sb')
            nc.sync.dma_start(out=qa_sb, in_=qa_own[:])
            kva_sb = attp.tile([P, NDT, KVAC], bf16, name='kva_sb')
            nc.sync.dma_start(out=kva_sb, in_=kva_own[:])
            kvb_sb = attp.tile([P, NKVT, HPC * (DN + DV)], bf16, name='kvb_sb')
            nc.sync.dma_start(out=kvb_sb, in_=kvb_own[:])

            # ============ phase 1: a-projections + input-norm stats =======
            with ExitStack() as ph1_scope:
                hkp = ph1_scope.enter_context(tc.tile_pool(name='hkp', bufs=1))
                hk = hkp.tile([P, NDT, S], bf16, name='hk')
                G1 = 4
                for g in range(NDT // G1):
                    nc.sync.dma_start(out=hk[:, g * G1:(g + 1) * G1, :],
                                      in_=hT[:, g * G1:(g + 1) * G1, :])
                ph1 = ph1_scope.enter_context(tc.tile_pool(name='ph1', bufs=1))
                ph1ps = ph1_scope.enter_context(
                    tc.tile_pool(name='ph1ps', bufs=1, space='PSUM'))
                # lq GEMMs per chunk + AG  (PSUM: lq1 2 + lq2 1 = 3 banks)
                for c in range(NCH):
                    cs = ts(c, TCH)
                    ps1 = ph1ps.tile([P, TCH], f32, tag='lq1', bufs=2, name='lq1')
                    ps2 = ph1ps.tile([QAC - P, TCH], f32, tag='lq2', bufs=1,
                                     name='lq2')
                    for k in range(NDT):
                        mm(ps1, qa_sb[:, k, 0:P], hk[:, k, cs], k == 0, k == NDT - 1)
                        mm(ps2, qa_sb[:, k, P:QAC], hk[:, k, cs], k == 0, k == NDT - 1)
                    lq1 = ph1.tile([P, TCH], bf16, tag='lq1s', bufs=2, name='lq1s')
                    nc.vector.tensor_copy(lq1, ps1)
                    nc.sync.dma_start(out=lq_dram[c][0:P, :], in_=lq1)
                    lq2 = ph1.tile([QAC - P, TCH], bf16, tag='lq2s', bufs=2,
                                   name='lq2s')
                    nc.vector.tensor_copy(lq2, ps2)
                    nc.sync.dma_start(out=lq_dram[c][P:QAC, :], in_=lq2)
                    ag(lq_dram[c], lq_ag[c])
                    # first half of the square-accumulate chains fills DVE
                    # idle time before the first drain
                    if c == 0:
                        ss_accs = sq_chains(lambda k: hk[:, k, :], NDT, S, ph1,
                                            'ss1', 2, 0, 16)
                sq_chains(lambda k: hk[:, k, :], NDT, S, ph1, 'ss1', 2, 16, NDT,
                          accs=ss_accs)
                # lkv GEMMs per chunk + AG  (PSUM: 1 bank)
                for c in range(NCH):
                    cs = ts(c, TCH)
                    pskv = ph1ps.tile([KVAC, TCH], f32, tag='lkv', bufs=1,
                                      name='lkv')
                    for k in range(NDT):
                        mm(pskv, kva_sb[:, k, :], hk[:, k, cs], k == 0, k == NDT - 1)
                    lkv1 = ph1.tile([KVAC, TCH], bf16, tag='lkvs', bufs=2,
                                    name='lkvs')
                    nc.vector.tensor_copy(lkv1, pskv)
                    nc.sync.dma_start(out=lkv_dram[c][:], in_=lkv1)
                    ag(lkv_dram[c], lkv_ag[c])
                # input-norm factors per chunk (PSUM: ss 1 + bc 1 = 2 banks)
                for c in range(NCH):
                    cs = ts(c, TCH)
                    ss = sq_reduce(ss_accs, cs, ph1, ph1ps, 'ss_ps', ps_bufs=2)
                    r1 = finish_norm(ss, 1.0 / D, f'r1_{c}', tag='r1')
                    r1sq = vecs.tile([1, TCH], f32, tag='r1sq', bufs=2,
                                     name='r1sq')
                    nc.vector.tensor_mul(r1sq, r1, r1)
                    r1_c[c] = (r1, r1sq)
                    r1b = bcast_row(r1, f'r1_{c}', ph1, ph1ps, ps_bufs=2, bufs=1)
                    cr = attp.tile([P, TCH], bf16, tag='cosr1', bufs=2,
                                   name='cosr1')
                    nc.vector.tensor_mul(cr, cos_sb[:, cs], r1b)
                    sr = attp.tile([P, TCH], bf16, tag='sinr1', bufs=2,
                                   name='sinr1')
                    nc.vector.tensor_mul(sr, sin_sb[:, cs], r1b)
                    cosr1_c[c], sinr1_c[c] = cr, sr

            # o_proj weights resident; loaded during phases 2-4
            o_pool = att_scope.enter_context(tc.tile_pool(name='o_pool', bufs=1))
            o_sb = o_pool.tile([P, NDT, OC], bf16, name='o_sb')
            for g in range(4):
                nc.sync.dma_start(
                    out=o_sb[:, g * 8:(g + 1) * 8, :],
                    in_=o_own[:, ds(g * 8 * OC, 8 * OC)]
                    .rearrange('p (k n) -> p k n', n=OC))

            # ============ phases 2-4 per chunk ============================
            pre = att_scope.enter_context(tc.tile_pool(name='pre', bufs=1))
            for c in range(NCH):
                cs = ts(c, TCH)
                r1, r1sq = r1_c[c]
                with tc.tile_pool(name='ph2', bufs=1) as ph2, \
                     tc.tile_pool(name='ph2w', bufs=3) as ph2w, \
                     tc.tile_pool(name='ph2ps', bufs=1, space='PSUM') as ph2ps:
                    lqn = pre.tile([P, NQLT, TCH], bf16, tag='lqn', bufs=1,
                                   name='lqn')
                    for g in range(2):
                        nc.sync.dma_start(
                            out=lqn[:, g * 6:(g + 1) * 6, :],
                            in_=lq_ag[c].rearrange('(k p) s -> p k s', p=P)
                            [:, g * 6:(g + 1) * 6, :])
                    kvn = pre.tile([P, NKVT, TCH], bf16, tag='kvn', bufs=1,
                                   name='kvn')
                    nc.sync.dma_start(
                        out=kvn, in_=lkv_ag[c][0:KVLORA, :]
                        .rearrange('(k p) s -> p k s', p=P))

                    # q_b GEMM mc 0-2 (PSUM qb_ps: 3 banks)
                    def qbw_tile(mc2):
                        w = ph2w.tile([P, NQLT, P], bf16, tag='qbw', bufs=6,
                                      name='qbw')
                        nc.sync.dma_start(
                            out=w, in_=qb_own[ds(mc2 * P, P), :]
                            .rearrange('p (k n) -> p k n', n=P))
                        return w

                    ps_q = []
                    for mc2 in range(3):
                        w = qbw_tile(mc2)
                        ps = ph2ps.tile([P, TCH], f32, tag='qb_ps', bufs=3,
                                        name='qb_ps')
                        for k in range(NQLT):
                            mm(ps, w[:, k, :], lqn[:, k, :],
                               k == 0, k == NQLT - 1)
                        ps_q.append(ps)
                    # rq stats + fq factor (PSUM st 1 + bc 1 = 2 banks)
                    acc_q = sq_chains(lambda k: lqn[:, k, :], NQLT, TCH, ph2,
                                      'st2', 2)
                    ssq = sq_reduce(acc_q, None, ph2, ph2ps, 'st_ps')
                    rq = finish_norm(ssq, 1.0 / QLORA, f'rq_{c}', extra_sq=r1sq)
                    fq = vrow(f'fq_{c}')
                    nc.vector.tensor_mul(fq, rq, r1)
                    fq_b = bcast_row(fq, f'fq_{c}', ph2, ph2ps)
                    cf = ph2.tile([P, TCH], bf16, tag='cosfq', bufs=1,
                                  name='cosfq')
                    nc.vector.tensor_mul(cf, cos_sb[:, cs], fq_b)
                    sf = ph2.tile([P, TCH], bf16, tag='sinfq', bufs=1,
                                  name='sinfq')
                    nc.vector.tensor_mul(sf, sin_sb[:, cs], fq_b)
                    for mc2 in range(3):
                        nc.vector.tensor_mul(qT[:, mc2, cs], ps_q[mc2], fq_b)
                    # remaining q_b tiles: mc 3 (nope) + 4,5 (pe with rope)
                    # (PSUM qrot: 1 bank)
                    for mc2 in range(3, NQB):
                        w = qbw_tile(mc2)
                        ps = ph2ps.tile([P, TCH], f32, tag='qb_ps', bufs=3,
                                        name='qb_ps')
                        for k in range(NQLT):
                            mm(ps, w[:, k, :], lqn[:, k, :],
                               k == 0, k == NQLT - 1)
                        if mc2 == 3:
                            nc.vector.tensor_mul(qT[:, mc2, cs], ps, fq_b)
                        else:
                            qraw = ph2.tile([P, TCH], bf16, tag='qraw', bufs=1,
                                            name='qraw')
                            nc.vector.tensor_copy(qraw, ps)
                            ps2 = ph2ps.tile([P, TCH], f32, tag='qrot', bufs=1,
                                             name='qrot')
                            nc.tensor.matmul(ps2, rot2_sb, qraw,
                                             start=True, stop=True)
                            rot_s = ph2.tile([P, TCH], f32, tag='rot_qs',
                                             bufs=1, name='rot_qs')
                            nc.vector.tensor_mul(rot_s, ps2, sf)
                            nc.vector.tensor_mul(qT[:, mc2, cs], qraw, cf)
                            nc.vector.tensor_add(qT[:, mc2, cs],
                                                 qT[:, mc2, cs], rot_s)

                    # kv stats + fkv
                    acc_kv = sq_chains(lambda k: kvn[:, k, :], NKVT, TCH, ph2,
                                       'st2', 2)
                    sskv = sq_reduce(acc_kv, None, ph2, ph2ps, 'st_ps')
                    rkv = finish_norm(sskv, 1.0 / KVLORA, f'rkv_{c}',
                                      extra_sq=r1sq)
                    fkv = vrow(f'fkv_{c}')
                    nc.vector.tensor_mul(fkv, rkv, r1)
                    fkv_b = bcast_row(fkv, f'fkv_{c}', ph2, ph2ps)
                    # kT on raw kvn, drain-scaled (PSUM kv_ps: 2 banks)
                    for j in range(HPC):
                        ps = ph2ps.tile([P, TCH], f32, tag='kv_ps', bufs=2,
                                        name='kv_ps')
                        for k in range(NKVT):
                            mm(ps, kvb_sb[:, k, ts(j, DN)], kvn[:, k, :],
                               k == 0, k == NKVT - 1)
                        nc.vector.tensor_mul(kT[:, j, cs], ps, fkv_b)
                    # scale kvn in place for the v GEMM
                    for k in range(NKVT):
                        nc.vector.tensor_mul(kvn[:, k, :], kvn[:, k, :], fkv_b)
                    for i in range(4 * c, 4 * c + 4):
                        il = i - 4 * c
                        ps = ph2ps.tile([P, HPC * DV], f32, tag='kv_ps', bufs=2,
                                        name='kv_ps')
                        for k in range(NKVT):
                            mm(ps, kvn[:, k, ts(il, P)],
                               kvb_sb[:, k, HPC * DN:], k == 0, k == NKVT - 1)
                        nc.vector.tensor_copy(v_sb[:, i, :], ps)
                    # k_pe rope: kpe = raw*(cos*r1) + rot(raw)*(sin*r1)
                    kpe_raw = ph2.tile([DR, TCH], bf16, tag='kpe_raw', bufs=1,
                                       name='kpe_raw')
                    nc.sync.dma_start(out=kpe_raw,
                                      in_=lkv_ag[c][KVLORA:KVLORA + DR, :])
                    ps_rot = ph2ps.tile([P, TCH], f32, tag='qrot', bufs=1,
                                        name='kperot')
                    nc.tensor.matmul(ps_rot[0:DR, :], rot2_sb[0:DR, 0:DR],
                                     kpe_raw, start=True, stop=True)
                    rot_s = ph2.tile([DR, TCH], f32, tag='kpe_rs', bufs=1,
                                     name='kpe_rs')
                    nc.vector.tensor_mul(rot_s, ps_rot[0:DR, :],
                                         sinr1_c[c][0:DR, :])
                    kpe_t = ph2.tile([DR, TCH], f32, tag='kpe_t', bufs=1,
                                     name='kpe_t')
                    nc.vector.tensor_mul(kpe_t, kpe_raw, cosr1_c[c][0:DR, :])
                    nc.vector.tensor_add(kpe[0:DR, cs], kpe_t, rot_s)
                    nc.sync.dma_start(out=kpe[DR:P, cs], in_=kpe[0:DR, cs])

                # ---- attention for this chunk ----------------------------
                # PSUM: sc 2 + se 2 + at 2 + bc 2 = 8 banks
                with tc.tile_pool(name='ph4', bufs=1) as ph4, \
                     tc.tile_pool(name='ph4p', bufs=1) as ph4p, \
                     tc.tile_pool(name='ph4ps', bufs=1, space='PSUM') as ph4ps:
                    ilist = list(range(4 * c + 4))
                    for j in range(HPC):
                        pe_mc = HPC * DN // P + (j * DR) // P
                        pe_off = (j * DR) % P
                        probs = []
                        for i in ilist:
                            ps = ph4ps.tile([P, TCH], f32, tag='sc_ps', bufs=2,
                                            name='sc_ps')
                            mm(ps, kT[:, j, ts(i, P)], qT[:, j, cs], True, False)
                            mm(ps, kpe[pe_off:pe_off + DR, ts(i, P)],
                               qT[pe_off:pe_off + DR, pe_mc, cs], False, True)
                            e = ph4p.tile([P, TCH], bf16, tag=f'probs{i}',
                                          bufs=1, name=f'probs{i}')
                            nc.scalar.activation(e, ps, AF.Exp, scale=SCALE)
                            if i // 4 == c:
                                nc.vector.tensor_mul(e, e, masks_sb[:, i % 4, :])
                            probs.append(e)
                        ps_se = ph4ps.tile([1, TCH], f32, tag='se_ps', bufs=2,
                                           name='se_ps')
                        for n, i in enumerate(ilist):
                            mm(ps_se, ones_bf, probs[n], n == 0,
                               n == len(ilist) - 1)
                        ps_at = ph4ps.tile([P, TCH], f32, tag='at_ps', bufs=2,
                                           name='at_ps')
                        for n, i in enumerate(ilist):
                            mm(ps_at, v_sb[:, i, ts(j, DV)], probs[n],
                               n == 0, n == len(ilist) - 1)
                        recip = vrow(f'recip_{c}_{j}')
                        sef = vrow(f'se_{c}_{j}')
                        nc.vector.tensor_copy(sef, ps_se)
                        nc.vector.reciprocal_approx_fast(out=recip, in_=sef)
                        recip_b = bcast_row(recip, 'recip', ph4, ph4ps,
                                            ps_bufs=2, bufs=1)
                        a = ph4.tile([P, TCH], bf16, tag='attn_o', bufs=2,
                                     name='attn_o')
                        nc.vector.tensor_mul(a, ps_at, recip_b)
                        nc.sync.dma_start(out=attn_dram[c][ts(j, DV), :], in_=a)
                ag(attn_dram[c], attn_ag[c])

            # ============ phase 5: o_proj + residual, per chunk ============
            with tc.tile_pool(name='ph5', bufs=1) as ph5, \
                 tc.tile_pool(name='ph5r', bufs=1) as ph5r, \
                 tc.tile_pool(name='ph5ps', bufs=1, space='PSUM') as ph5ps:
                hres_sb = ph5r.tile([P, NOB, S], f32, name='hres_sb')
                nc.sync.dma_start(out=hres_sb,
                                  in_=h_ownD.rearrange('(m p) s -> p m s', p=P))
                for c in range(NCH):
                    cs = ts(c, TCH)
                    att_rs = ph5r.tile([P, NDT, TCH], bf16, tag='att_rs', bufs=1,
                                       name='att_rs')
                    for g in range(8):
                        nc.sync.dma_start(
                            out=att_rs[:, g * 4:(g + 1) * 4, :],
                            in_=attn_ag[c].rearrange('(k p) s -> p k s', p=P)
                            [:, g * 4:(g + 1) * 4, :])
                    ps_o = [ph5ps.tile([P, TCH], f32, tag=f'o_ps{m2}', bufs=1,
                                       name=f'o_ps{m2}') for m2 in range(NOB)]
                    for k in range(NDT):
                        for mcc in range(NOB):
                            mm(ps_o[mcc], o_sb[:, k, ts(mcc, P)], att_rs[:, k, :],
                               k == 0, k == NDT - 1)
                    for mcc in range(NOB):
                        nc.vector.tensor_add(h2_own_sb[:, mcc, cs], ps_o[mcc],
                                             hres_sb[:, mcc, cs])
                        h2b = ph5.tile([P, TCH], bf16, tag='h2b', bufs=2,
                                       name='h2b')
                        nc.vector.tensor_copy(h2b, h2_own_sb[:, mcc, cs])
                        nc.sync.dma_start(out=h2_dram[c][ts(mcc, P), :], in_=h2b)
                    ag(h2_dram[c], h2_ag[c])

        # ============ phase 6: post-norm stats + gate/up (lag pipeline) ====
        with ExitStack() as mlp_scope:
            mlp_sb = mlp_scope.enter_context(tc.tile_pool(name='mlp_sb', bufs=1))
            h2T = mlp_sb.tile([P, NDT, S], bf16, name='h2T')
            ph6 = mlp_scope.enter_context(tc.tile_pool(name='ph6', bufs=1))
            ph6w = mlp_scope.enter_context(tc.tile_pool(name='ph6w', bufs=1))
            ph6ps = mlp_scope.enter_context(
                tc.tile_pool(name='ph6ps', bufs=1, space='PSUM'))
            for c in range(NCH):
                cs = ts(c, TCH)
                for g in range(8):
                    nc.sync.dma_start(
                        out=h2T[:, g * 4:(g + 1) * 4, cs],
                        in_=h2_ag[c].rearrange('(k p) s -> p k s', p=P)
                        [:, g * 4:(g + 1) * 4, :])

            def stats6(c):
                cs = ts(c, TCH)
                acc2 = sq_chains(lambda k: h2T[:, k, cs], NDT, TCH, ph6,
                                 'ss2', 2)
                ss2 = sq_reduce(acc2, None, ph6, ph6ps, 'st_ps')
                r2 = finish_norm(ss2, 1.0 / D, f'r2_{c}')
                r2_b_c[c] = bcast_row(r2, f'r2_{c}', mlp_sb, ph6ps)

            stats6(0)
            # job order: 3-tile c0 prologue, then interleave c1 at lag 2
            LAG = 2
            jobs = []
            for mcc in range(NMC):
                jobs.append((mcc, 0))
                if mcc >= LAG:
                    jobs.append((mcc - LAG, 1))
            for mcc in range(NMC - LAG, NMC):
                jobs.append((mcc, 1))
            woff = [mcc * NDT * P for mcc in range(NMC)]  # col offsets (els)
            wtiles = {}
            for mcc, c in jobs:
                if c == 1 and r2_b_c[1] is None:
                    stats6(1)
                cs = ts(c, TCH)
                rows = min(P, IC - mcc * P)
                if mcc not in wtiles:
                    wg = ph6w.tile([P, NDT, P], bf16, tag='wg', bufs=3,
                                   name='wg')
                    wu = ph6w.tile([P, NDT, P], bf16, tag='wu', bufs=3,
                                   name='wu')
                    for wt, wsrc in ((wg, gate_own), (wu, up_own)):
                        for hh in range(2):
                            nc.sync.dma_start(
                                out=wt[:, hh * 16:(hh + 1) * 16, 0:rows],
                                in_=wsrc[:, ds(woff[mcc] + hh * 16 * rows,
                                               16 * rows)]
                                .rearrange('p (k n) -> p k n', n=rows))
                    wtiles[mcc] = (wg, wu)
                wg, wu = wtiles[mcc]
                ps_g = ph6ps.tile([P, TCH], f32, tag='g_ps', bufs=3, name='g_ps')
                ps_u = ph6ps.tile([P, TCH], f32, tag='u_ps', bufs=3, name='u_ps')
                for k in range(NDT):
                    mm(ps_g[0:rows], wg[:, k, 0:rows], h2T[:, k, cs],
                       k == 0, k == NDT - 1)
                    mm(ps_u[0:rows], wu[:, k, 0:rows], h2T[:, k, cs],
                       k == 0, k == NDT - 1)
                g = ph6.tile([P, TCH], f32, tag='g_sb', bufs=2, name='g_sb')
                nc.vector.tensor_mul(g[0:rows], ps_g[0:rows],
                                     r2_b_c[c][0:rows])
                nc.scalar.activation(g[0:rows], g[0:rows], AF.Silu)
                u = ph6.tile([P, TCH], f32, tag='u_sb', bufs=2, name='u_sb')
                nc.vector.tensor_mul(u[0:rows], ps_u[0:rows],
                                     r2_b_c[c][0:rows])
                m = ph6.tile([P, TCH], bf16, tag='m_sb', bufs=2, name='m_sb')
                nc.vector.tensor_mul(m[0:rows], g[0:rows], u[0:rows])
                if mcc * P < MA:
                    nc.sync.dma_start(out=m_dramA[ds(mcc * P, rows), cs],
                                      in_=m[0:rows])
                else:
                    nc.sync.dma_start(out=m_dramB[ds(mcc * P - MA, rows), cs],
                                      in_=m[0:rows])
                if (mcc, c) == (MA // P - 1, 1):
                    ag(m_dramA, m_agA)
            ag(m_dramB, m_agB)

        # ============ phase 7: down_proj + final residual ============
        with tc.tile_pool(name='ph7', bufs=1) as ph7, \
             tc.tile_pool(name='ph7ps', bufs=1, space='PSUM') as ph7ps:
            ps_d = [ph7ps.tile([P, S], f32, tag=f'd_ps{m2}', bufs=1,
                               name=f'd_ps{m2}') for m2 in range(NOB)]
            G7 = 2
            kglob = 0
            for half, (src_ag, ntiles) in enumerate(
                    ((m_agA, NC * MA // P), (m_agB, NC * MB // P))):
                for g in range(ntiles // G7):
                    mk = ph7.tile([P, G7, S], bf16, tag='mk', bufs=6, name='mk')
                    nc.sync.dma_start(
                        out=mk, in_=src_ag[g * G7 * P:(g + 1) * G7 * P, :]
                        .rearrange('(k p) s -> p k s', p=P))
                    w = ph7.tile([P, G7, OC], bf16, tag='dw', bufs=6, name='dw')
                    nc.sync.dma_start(
                        out=w, in_=down_own[:, ds((kglob + g * G7) * OC, G7 * OC)]
                        .rearrange('p (k n) -> p k n', n=OC))
                    for kk in range(G7):
                        k = kglob + g * G7 + kk
                        for mcc in range(NOB):
                            for cc in range(NCH):
                                cc_s = ts(cc, TCH)
                                mm(ps_d[mcc][:, cc_s], w[:, kk, ts(mcc, P)],
                                   mk[:, kk, cc_s], k == 0, k == NIT - 1)
                kglob += ntiles
            for mcc in range(NOB):
                o = ph7.tile([P, S], f32, tag='o_out', bufs=2, name='o_out')
                nc.vector.tensor_add(o, ps_d[mcc], h2_own_sb[:, mcc, :])
                nc.sync.dma_start(out=out[ts(mcc, P), :], in_=o)

    nc.compile()
    return nc


def _tileize(w, cols_slice=None):
    """[D_in, n] -> [P, D_in//P, n] contiguous bf16."""
    if cols_slice is not None:
        w = w[:, cols_slice]
    kin = w.shape[0] // P
    return np.ascontiguousarray(
        w.reshape(kin, P, w.shape[1]).transpose(1, 0, 2)).astype(BF16)


def _flat(w, cols_slice=None):
    """[D_in, n] -> [P, (D_in//P)*n] flat k-major blocks."""
    t = _tileize(w, cols_slice)
    return np.ascontiguousarray(t.reshape(P, -1))


def _prep_inputs(inputs):
    h = np.ascontiguousarray(np.asarray(inputs['hidden_states'], np.float32))
    hT = np.ascontiguousarray(h.T)
    cosT = np.ascontiguousarray(np.asarray(inputs['cos'], np.float32).T)
    sinT = np.ascontiguousarray(np.asarray(inputs['sin'], np.float32).T)
    q_a_w = np.asarray(inputs['q_a_w'], np.float32)
    q_b_w = np.asarray(inputs['q_b_w'], np.float32)
    kv_a_w = np.asarray(inputs['kv_a_w'], np.float32)
    kv_b_w = np.asarray(inputs['kv_b_w'], np.float32)
    o_w = np.asarray(inputs['o_w'], np.float32)
    gate_w = np.asarray(inputs['gate_w'], np.float32)
    up_w = np.asarray(inputs['up_w'], np.float32)
    down_w = np.asarray(inputs['down_w'], np.float32)

    pidx = np.arange(P)[:, None]
    cidx = np.arange(TCH)[None, :]
    masks = np.stack([(cidx - pidx >= P * k) for k in range(4)]).astype(BF16)

    cosT2 = np.ascontiguousarray(np.vstack([cosT, cosT]))
    sinT2 = np.ascontiguousarray(np.vstack([sinT, sinT]))
    R = np.zeros((DR, DR), np.float32)
    R[np.arange(DR // 2), np.arange(DR // 2) + DR // 2] = -1.0
    R[np.arange(DR // 2) + DR // 2, np.arange(DR // 2)] = 1.0
    R2 = np.zeros((P, P), np.float32)
    R2[:DR, :DR] = R
    R2[DR:, DR:] = R
    rot2T = np.ascontiguousarray(R2.T)

    m_row_order = np.concatenate(
        [np.arange(MA) + rr * IC for rr in range(NC)] +
        [np.arange(MA, IC) + rr * IC for rr in range(NC)])

    hT_tiled = _tileize(hT)

    def gup_flat(w, r):
        """per-mc [P, NDT*rows] blocks concatenated along the free axis."""
        blocks = []
        for mcc in range(NMC):
            rows = min(P, IC - mcc * P)
            blocks.append(_flat(w, np.s_[r * IC + mcc * P:
                                         r * IC + mcc * P + rows]))
        return np.ascontiguousarray(np.concatenate(blocks, axis=1))

    in_maps = []
    for r in range(NC):
        heads = range(r * HPC, (r + 1) * HPC)
        qb_cols = np.concatenate(
            [q_b_w[:, hh * (DN + DR):hh * (DN + DR) + DN] for hh in heads] +
            [q_b_w[:, hh * (DN + DR) + DN:(hh + 1) * (DN + DR)] for hh in heads],
            axis=1)
        kvb_cols = np.concatenate(
            [kv_b_w[:, hh * (DN + DV):hh * (DN + DV) + DN] for hh in heads] +
            [kv_b_w[:, hh * (DN + DV) + DN:(hh + 1) * (DN + DV)] for hh in heads],
            axis=1)
        # q_b as [NQB*P, NQLT*P]: per-mc [P, NQLT*P] flat blocks stacked
        qb_blocks = np.stack([_flat(qb_cols, np.s_[mc * P:(mc + 1) * P])
                              for mc in range(NQB)])
        in_maps.append({
            'hT': hT_tiled,
            'h_ownD': np.ascontiguousarray(hT[r * OC:(r + 1) * OC]),
            'qa_own': _tileize(q_a_w, np.s_[r * QAC:(r + 1) * QAC]),
            'kva_own': _tileize(kv_a_w, np.s_[r * KVAC:(r + 1) * KVAC]),
            'qb_own': np.ascontiguousarray(
                qb_blocks.reshape(NQB * P, NQLT * P)),
            'kvb_own': _tileize(kvb_cols),
            'o_own': _flat(o_w, np.s_[r * OC:(r + 1) * OC]),
            'gate_own': gup_flat(gate_w, r),
            'up_own': gup_flat(up_w, r),
            'down_own': _flat(
                np.ascontiguousarray(down_w[m_row_order,
                                            r * OC:(r + 1) * OC])),
            'cosT2': cosT2,
            'sinT2': sinT2,
            'rot2T': rot2T.astype(BF16),
            'masks': masks,
        })
    return in_maps


def kernel(**inputs) -> np.ndarray:
    if 'nc' not in _CACHE:
        _CACHE['nc'] = _build()
    nc = _CACHE['nc']
    from concourse.bass_utils import run_bass_kernel_spmd
    in_maps = _prep_inputs(inputs)
    res = run_bass_kernel_spmd(nc, in_maps, core_ids=list(range(NC)))
    outT = np.concatenate([res.results[r]['out'] for r in range(NC)], axis=0)
    return np.ascontiguousarray(outT.T)
